# revision 1
# baseline (speedup 1.0000x reference)
"""Trainium2 Bass kernel for a dense transformer decoder block.

Strategy (8 NeuronCores, tensor-parallel a la Megatron):
  - heads sharded across cores (H/8 heads each) for attention,
    FFN hidden dim sharded (HID/8 each).
  - Activations kept in transposed layout [D, tokens] on device so every
    matmul contracts over the partition dim with fp32r (full-rate fp32).
  - rmsnorm column sums via ones-matmul on the PE (replicated [128, t]
    stats, so no partition broadcasts are needed).
  - Residual x is folded into the wo AllReduce as x/8; the AllReduce
    output IS h.  The final residual h is folded into the w2
    ReduceScatter as h/8; the RS output IS the final result, sharded
    over D rows across cores.  Host reassembles + transposes.
  - Causality is not hardcoded: the mask input is classified host-side
    into skip / plain / mixed 128x512 blocks; mixed tiles are shipped
    as constants (4 distinct tiles for a causal mask).
"""

import os
import sys

try:  # the axon sitecustomize usually provides concourse already
    import concourse.bass  # noqa: F401
except ImportError:  # pragma: no cover
    sys.path.insert(0, "/opt/trn_rl_repo")

from contextlib import ExitStack

import ml_dtypes
import numpy as np

import concourse.bacc as bacc
import concourse.tile as tile
from concourse import mybir
from concourse.bass_utils import run_bass_kernel_spmd
from concourse.masks import make_identity

F32 = mybir.dt.float32
F32R = mybir.dt.float32r
BF16 = mybir.dt.bfloat16
N_CORES = 8
P = 128
QW = 512  # q-tile / token-tile width
EPS = 1e-6
AF = mybir.ActivationFunctionType
BF16_NP = ml_dtypes.bfloat16


def ts(i, w):
    return slice(i * w, (i + 1) * w)


def _classify_mask(mask, S):
    """mask: [S, S] additive (q, k). Returns (table, tiles).
    table[(kt, j)] = 'skip' | 'plain' | int mask-tile index.
    tiles: list of [128, QW] float32 arrays in scoresT ([k, q]) layout."""
    table = {}
    tiles = []
    keys = {}
    for j in range(S // QW):
        for kt in range(S // P):
            sub = mask[ts(j, QW), ts(kt, P)]  # [q, k]
            if np.all(sub <= -1e8):
                table[(kt, j)] = "skip"
            elif np.all(sub == 0.0):
                table[(kt, j)] = "plain"
            else:
                t = np.ascontiguousarray(sub.T.astype(np.float32))  # [k, q]
                key = t.tobytes()
                if key not in keys:
                    keys[key] = len(tiles)
                    tiles.append(t)
                table[(kt, j)] = keys[key]
    return table, tiles


def build_program(B, S, D, H, HID, mask_table, n_mask):
    HD = 128
    assert D == (D // P) * P and H * HD == D
    HPC = H // N_CORES            # heads per core
    assert HPC * N_CORES == H
    C = D // P                    # contraction chunks over D
    S_TILES = S // QW             # q tiles per batch
    KT = S // P                   # k tiles per batch
    T = B * S                     # total tokens
    OC = HPC * HD // P            # wo input-channel chunks (== HPC)
    HIDC = HID // N_CORES // P    # hidden tiles per core
    HC = T // QW                  # half-chunk count (512-token tiles)
    N_CHUNKS = max(1, T // 1024)  # collective chunks
    CH_T = T // N_CHUNKS          # tokens per collective chunk
    DS = D // N_CORES             # output row shard per core

    nc = bacc.Bacc(trn_type="TRN2", num_devices=N_CORES)

    xt = nc.dram_tensor("xt", [B, D, S], F32, kind="ExternalInput").ap()
    wq = nc.dram_tensor("wq", [C, P, HPC * HD], BF16, kind="ExternalInput").ap()
    wk = nc.dram_tensor("wk", [C, P, HPC * HD], BF16, kind="ExternalInput").ap()
    wv = nc.dram_tensor("wv", [C, P, HPC * HD], BF16, kind="ExternalInput").ap()
    wo = nc.dram_tensor("wo", [OC, P, D], BF16, kind="ExternalInput").ap()
    w1 = nc.dram_tensor("w1", [C, P, HIDC * P], BF16, kind="ExternalInput").ap()
    w2 = nc.dram_tensor("w2", [HIDC, P, D], BF16, kind="ExternalInput").ap()
    mk = None
    if n_mask:
        mk = nc.dram_tensor("mk", [n_mask, P, QW], F32, kind="ExternalInput").ap()

    ar_in = [nc.dram_tensor(f"ar_in{k}", [D, CH_T], F32) for k in range(N_CHUNKS)]
    ar_out = [
        nc.dram_tensor(f"ar_out{k}", [D, CH_T], F32, addr_space="Shared")
        for k in range(N_CHUNKS)
    ]
    rs_in = [nc.dram_tensor(f"rs_in{k}", [D, CH_T], F32) for k in range(N_CHUNKS)]
    rs_out = [
        nc.dram_tensor(f"rs_out{k}", [DS, CH_T], F32) for k in range(N_CHUNKS)
    ]
    outs = [
        nc.dram_tensor(f"out{k}", [DS, CH_T], F32, kind="ExternalOutput")
        for k in range(N_CHUNKS)
    ]

    groups = [list(range(N_CORES))]

    def rb(ap):  # reinterpret an f32 DRAM source as f32r for DMA into f32r tiles
        return ap.bitcast(F32R)

    with tile.TileContext(nc) as tc, ExitStack() as ctx:
        const = ctx.enter_context(tc.tile_pool(name="const", bufs=1))
        stats = ctx.enter_context(tc.tile_pool(name="stats", bufs=2))
        sqp = ctx.enter_context(tc.tile_pool(name="sq", bufs=2))
        evp = ctx.enter_context(tc.tile_pool(name="ev", bufs=2))
        psum = ctx.enter_context(tc.tile_pool(name="psum", bufs=1, space="PSUM"))

        ones_f32 = const.tile([P, P], F32)
        nc.vector.memset(ones_f32[:], 1.0)
        ones = const.tile([P, P], BF16)
        nc.vector.tensor_copy(ones[:], ones_f32[:])
        eps_p1 = const.tile([P, 1], F32)
        nc.vector.memset(eps_p1[:], EPS)
        ident = const.tile([P, P], F32)
        make_identity(nc, ident[:])

        # ---------------- attention weights (resident) ----------------
        with tc.tile_pool(name="wqkv", bufs=1) as wqkvp, \
             tc.tile_pool(name="xa", bufs=2) as xap, \
             tc.tile_pool(name="qkv", bufs=1) as qkvp, \
             tc.tile_pool(name="exp", bufs=2) as expp, \
             tc.tile_pool(name="attn", bufs=1) as attp:
            mtiles = None
            if n_mask:
                mtiles = wqkvp.tile([P, n_mask, QW], F32, tag="mk")
                nc.sync.dma_start(mtiles[:], mk.rearrange("n p q -> p n q"))
            wq_sb = wqkvp.tile([P, C, HPC * HD], BF16, tag="wq")
            nc.sync.dma_start(wq_sb[:], wq.rearrange("c p o -> p c o"))
            wk_sb = wqkvp.tile([P, C, HPC * HD], BF16, tag="wk")
            nc.sync.dma_start(wk_sb[:], wk.rearrange("c p o -> p c o"))
            wv_sb = wqkvp.tile([P, C, HPC * HD], BF16, tag="wv")
            nc.sync.dma_start(wv_sb[:], wv.rearrange("c p o -> p c o"))
            wo_sb = wqkvp.tile([P, OC, D], BF16, tag="wo")
            nc.sync.dma_start(wo_sb[:], wo.rearrange("c p o -> p c o"))

            for b in range(B):
                # Flash-style: for each 512-token q-tile, project q/k/v,
                # then immediately run causal attention (only past k-tiles
                # exist and only past ones are needed), then the wo
                # partial + x/8 fold while x is still resident in SBUF.
                kT = qkvp.tile([P, HPC, S], BF16, tag="kT")
                vN = qkvp.tile([P, KT, HPC * HD], BF16, tag="vN")
                for j in range(S_TILES):
                    xti = xap.tile([P, C, QW], F32, tag="xa")
                    nc.sync.dma_start(
                        xti[:],
                        xt[b].rearrange("(c p) t -> p c t", p=P)[:, :, ts(j, QW)],
                    )
                    xb = xap.tile([P, C, QW], BF16, tag="xb")
                    for c in range(C):
                        nc.gpsimd.tensor_copy(xb[:, c, :], xti[:, c, :])
                    # rmsnorm stats (replicated over partitions)
                    cs = psum.tile([P, QW], F32, tag="cs", bufs=1)
                    for c in range(C):
                        sq = sqp.tile([P, QW], BF16, tag="sq")
                        nc.scalar.activation(sq[:], xti[:, c, :], AF.Square)
                        nc.tensor.matmul(
                            cs[:], ones[:], sq[:], start=(c == 0), stop=(c == C - 1)
                        )
                    rms = stats.tile([P, QW], F32, tag="rms")
                    nc.scalar.activation(
                        rms[:], cs[:], AF.Sqrt, bias=eps_p1[:], scale=1.0 / D
                    )
                    rinv = stats.tile([P, QW], F32, tag="rinv")
                    nc.vector.reciprocal(rinv[:], rms[:])
                    # rinv transposed to token-partition layout for v scaling
                    # (PE transpose of the partition-replicated rinv block)
                    rcol = stats.tile([P, QW // P], F32, tag="rcol")
                    for sub in range(QW // P):
                        tp = psum.tile([P, P], F32, tag="score", bufs=2)
                        nc.tensor.transpose(tp[:], rinv[:, ts(sub, P)], ident[:])
                        nc.vector.tensor_copy(rcol[:, sub : sub + 1], tp[:, 0:1])
                    # q/k projections from RAW x; rinv folded in at eviction
                    qTs = qkvp.tile([P, HPC, QW], BF16, tag="qT", bufs=2)
                    for h in range(HPC):
                        for w_sb, dst in ((wq_sb, qTs), (wk_sb, kT)):
                            pp = psum.tile([P, QW], F32, tag="mm", bufs=3)
                            for c in range(C):
                                nc.tensor.matmul(
                                    pp[:],
                                    w_sb[:, c, ts(h, HD)],
                                    xb[:, c, :],
                                    start=(c == 0),
                                    stop=(c == C - 1),
                                )
                            if dst is qTs:
                                nc.vector.tensor_mul(qTs[:, h, :], pp[:], rinv[:])
                            else:
                                nc.vector.tensor_mul(
                                    kT[:, h, ts(j, QW)], pp[:], rinv[:]
                                )
                    # v in natural layout; rinv via per-partition scalar
                    for sub in range(QW // P):
                        pv = psum.tile([P, QW], F32, tag="mm", bufs=3)
                        for c in range(C):
                            nc.tensor.matmul(
                                pv[:, : HPC * HD],
                                xb[:, c, ts(sub, P)],
                                wv_sb[:, c, :],
                                start=(c == 0),
                                stop=(c == C - 1),
                            )
                        nc.vector.tensor_scalar_mul(
                            vN[:, j * (QW // P) + sub, :],
                            pv[:, : HPC * HD],
                            rcol[:, sub : sub + 1],
                        )

                    # -------- attention for q-tile j --------
                    attnT = attp.tile([P, HPC, QW], BF16, tag="attnT", bufs=2)
                    for h in range(HPC):
                        kts = [
                            kt for kt in range(KT) if mask_table[(kt, j)] != "skip"
                        ]
                        pa = psum.tile([P, QW], F32, tag="pv", bufs=1)
                        den = psum.tile([P, QW], F32, tag="stat", bufs=1)
                        n_k = len(kts)
                        exs = [None] * n_k

                        def _den_pv(i):
                            kt = kts[i]
                            nc.tensor.matmul(
                                den[:], ones[:], exs[i][:],
                                start=(i == 0), stop=(i == n_k - 1),
                            )
                            nc.tensor.matmul(
                                pa[:],
                                vN[:, kt, ts(h, HD)],
                                exs[i][:],
                                start=(i == 0),
                                stop=(i == n_k - 1),
                            )

                        for i, kt in enumerate(kts):
                            msc = psum.tile([P, QW], F32, tag="score", bufs=2)
                            nc.tensor.matmul(
                                msc[:],
                                kT[:, h, ts(kt, P)],
                                qTs[:, h, :],
                                start=True,
                                stop=True,
                            )
                            ex = expp.tile([P, QW], BF16, tag="exp")
                            mt = mask_table[(kt, j)]
                            if mt == "plain":
                                nc.scalar.activation(ex[:], msc[:], AF.Exp)
                            else:
                                nc.vector.tensor_add(ex[:], msc[:], mtiles[:, mt, :])
                                nc.scalar.activation(ex[:], ex[:], AF.Exp)
                            exs[i] = ex
                            if i > 0:
                                _den_pv(i - 1)
                        _den_pv(n_k - 1)
                        rec = stats.tile([P, QW], F32, tag="rms")
                        nc.vector.reciprocal(rec[:], den[:])
                        nc.vector.tensor_mul(attnT[:, h, :], pa[:], rec[:])

                    # -------- wo partial + x/8 -> AR input --------
                    g = b * S + j * QW  # global token offset
                    k = g // CH_T
                    off = g % CH_T
                    for ot in range(C):
                        po = psum.tile([P, QW], F32, tag="mm", bufs=3)
                        for oc in range(OC):
                            nc.tensor.matmul(
                                po[:],
                                wo_sb[:, oc, ts(ot, P)],
                                attnT[:, oc, :],
                                start=(oc == 0),
                                stop=(oc == OC - 1),
                            )
                        ev = evp.tile([P, QW], F32, tag="ev")
                        nc.vector.scalar_tensor_tensor(
                            ev[:], xti[:, ot, :], 1.0 / N_CORES, po[:],
                            op0=mybir.AluOpType.mult, op1=mybir.AluOpType.add,
                        )
                        nc.gpsimd.dma_start(
                            ar_in[k].ap()[ts(ot, P), off : off + QW], ev[:]
                        )
        for k in range(N_CHUNKS):
            nc.gpsimd.collective_compute(
                "AllReduce",
                mybir.AluOpType.add,
                replica_groups=groups,
                ins=[ar_in[k].ap().opt()],
                outs=[ar_out[k].ap().opt()],
            )

        # ---------------- FFN phase (h = ar_out) ----------------
        with tc.tile_pool(name="wffn", bufs=1) as wffnp, \
             tc.tile_pool(name="hf", bufs=2) as hfp, \
             tc.tile_pool(name="up", bufs=2) as upp:
            w1_sb = wffnp.tile([P, C, HIDC * P], BF16, tag="w1")
            for ht_i in range(HIDC):
                nc.sync.dma_start(
                    w1_sb[:, :, ts(ht_i, P)],
                    w1.rearrange("c p o -> p c o")[:, :, ts(ht_i, P)],
                )
            w2_sb = wffnp.tile([P, HIDC, D], BF16, tag="w2")
            nc.sync.dma_start(w2_sb[:], w2.rearrange("c p o -> p c o"))

            for hc in range(HC):
                k = hc * QW // CH_T
                off = (hc * QW) % CH_T
                ht = hfp.tile([P, C, QW], F32, tag="hf")
                nc.sync.dma_start(
                    ht[:],
                    ar_out[k].ap().rearrange("(c p) t -> p c t", p=P)[
                        :, :, off : off + QW
                    ],
                )
                # rmsnorm2 stats
                cs = psum.tile([P, QW], F32, tag="cs", bufs=1)
                for c in range(C):
                    sq = sqp.tile([P, QW], BF16, tag="sq")
                    nc.scalar.activation(sq[:], ht[:, c, :], AF.Square)
                    nc.tensor.matmul(
                        cs[:], ones[:], sq[:], start=(c == 0), stop=(c == C - 1)
                    )
                rms = stats.tile([P, QW], F32, tag="rms")
                nc.scalar.activation(
                    rms[:], cs[:], AF.Sqrt, bias=eps_p1[:], scale=1.0 / D
                )
                r2 = stats.tile([P, QW], F32, tag="rinv")
                nc.vector.reciprocal(r2[:], rms[:])
                hn = hfp.tile([P, C, QW], BF16, tag="hn")
                for c in range(C):
                    nc.vector.tensor_mul(hn[:, c, :], ht[:, c, :], r2[:])

                # up = relu(w1^T hn)
                up = upp.tile([P, HIDC, QW], BF16, tag="up")
                for ht_i in range(HIDC):
                    pu = psum.tile([P, QW], F32, tag="mm", bufs=3)
                    for c in range(C):
                        nc.tensor.matmul(
                            pu[:],
                            w1_sb[:, c, ts(ht_i, P)],
                            hn[:, c, :],
                            start=(c == 0),
                            stop=(c == C - 1),
                        )
                    nc.scalar.activation(up[:, ht_i, :], pu[:], AF.Relu)

                # down partial + h/8 -> RS input
                for ot in range(C):
                    pd = psum.tile([P, QW], F32, tag="mm", bufs=3)
                    for c in range(HIDC):
                        nc.tensor.matmul(
                            pd[:],
                            w2_sb[:, c, ts(ot, P)],
                            up[:, c, :],
                            start=(c == 0),
                            stop=(c == HIDC - 1),
                        )
                    ev = evp.tile([P, QW], F32, tag="ev")
                    nc.vector.scalar_tensor_tensor(
                        ev[:], ht[:, ot, :], 1.0 / N_CORES, pd[:],
                        op0=mybir.AluOpType.mult, op1=mybir.AluOpType.add,
                    )
                    nc.gpsimd.dma_start(rs_in[k].ap()[ts(ot, P), off : off + QW], ev[:])

        for k in range(N_CHUNKS):
            nc.gpsimd.collective_compute(
                "ReduceScatter",
                mybir.AluOpType.add,
                replica_groups=groups,
                ins=[rs_in[k].ap().opt()],
                outs=[rs_out[k].ap().opt()],
            )
            nc.sync.dma_start(outs[k].ap(), rs_out[k].ap())

    nc.compile()
    return nc, N_CHUNKS, CH_T, DS


_CACHE = {}
LAST_RESULT = None


def _get_program(B, S, D, H, HID, mask_table, n_mask, mask_key):
    key = (B, S, D, H, HID, mask_key)
    if key not in _CACHE:
        _CACHE[key] = build_program(B, S, D, H, HID, mask_table, n_mask)
    return _CACHE[key]


def kernel(x, mask, wq, wk, wv, wo, w1, w2, attn_norm_w, ffn_norm_w):
    x = np.asarray(x, dtype=np.float32)
    mask = np.asarray(mask, dtype=np.float32)
    wq, wk, wv, wo = (np.asarray(a, dtype=np.float32) for a in (wq, wk, wv, wo))
    w1, w2 = np.asarray(w1, dtype=np.float32), np.asarray(w2, dtype=np.float32)
    attn_norm_w = np.asarray(attn_norm_w, dtype=np.float32)
    ffn_norm_w = np.asarray(ffn_norm_w, dtype=np.float32)

    B, S, D = x.shape
    H = D // 128  # HD is fixed at 128 (= SBUF partition count)
    HID = w1.shape[0]
    HD = D // H
    HPC = H // N_CORES
    C = D // P
    HIDC = HID // N_CORES // P

    mask_table, mtiles_np = _classify_mask(
        np.broadcast_to(mask, (1, 1, S, S))[0, 0], S
    )
    mask_key = hash(tuple(sorted((k, str(v)) for k, v in mask_table.items())))
    nc, N_CHUNKS, CH_T, DS = _get_program(
        B, S, D, H, HID, mask_table, len(mtiles_np), mask_key
    )

    # ---- host-side prep ----
    xt = np.ascontiguousarray(x.transpose(0, 2, 1))  # [B, D, S]
    wq_f = (wq * attn_norm_w[None, :]) / np.sqrt(HD)
    wk_f = wk * attn_norm_w[None, :]
    wv_f = wv
    w1_f = w1 * ffn_norm_w[None, :]

    in_maps = []
    for c in range(N_CORES):
        hs = slice(c * HPC * HD, (c + 1) * HPC * HD)
        qs = np.ascontiguousarray(wq_f[hs].T).reshape(C, P, HPC * HD).astype(BF16_NP)
        ks = np.ascontiguousarray(wk_f[hs].T).reshape(C, P, HPC * HD).astype(BF16_NP)
        vs = np.ascontiguousarray(wv_f[hs].T).reshape(C, P, HPC * HD).astype(BF16_NP)
        os_ = np.ascontiguousarray(wo[:, hs].T).reshape(HPC, P, D).astype(BF16_NP)
        fs = slice(c * HIDC * P, (c + 1) * HIDC * P)
        w1s = np.ascontiguousarray(w1_f[fs].T).reshape(C, P, HIDC * P).astype(BF16_NP)
        # w2 shard -> [hid_c, p, o]
        w2r = (
            np.ascontiguousarray(w2[:, fs].T)
            .reshape(HIDC, P, D)
            .astype(BF16_NP)
        )
        m = {
            "xt": xt,
            "wq": qs,
            "wk": ks,
            "wv": vs,
            "wo": os_,
            "w1": w1s,
            "w2": w2r,
        }
        if len(mtiles_np):
            m["mk"] = np.stack(mtiles_np)
        in_maps.append(m)

    trace = os.environ.get("KTRACE", "0") == "1"
    res = run_bass_kernel_spmd(nc, in_maps, list(range(N_CORES)), trace=trace)
    global LAST_RESULT
    LAST_RESULT = res

    out_T = np.empty((D, B * S), dtype=np.float32)
    for r_ in range(N_CORES):
        for k in range(N_CHUNKS):
            out_T[r_ * DS : (r_ + 1) * DS, k * CH_T : (k + 1) * CH_T] = res.results[
                r_
            ][f"out{k}"]
    return np.ascontiguousarray(out_T.reshape(D, B, S).transpose(1, 2, 0))



# revision 2
# speedup vs baseline: 1.5786x; 1.5786x over previous
"""Trainium2 Bass kernel for a dense transformer decoder block.

Strategy (8 NeuronCores):
  - Attention tensor-parallel over heads (2 heads/core); activations in
    transposed layout [D, tokens]; all matmuls bf16 with fp32 PSUM.
  - wo partials (+ x/8 residual fold) are written in fp16 into
    token-scattered ReduceScatter buffers: 4 chunked RS collectives
    (1024 tokens each) issued mid-attention so they overlap compute.
    The RS output hands each core h for its own 512 tokens.
  - FFN is data-parallel: each core runs the FULL FFN on its 512 tokens,
    streaming the full w1/w2 (bf16) from HBM under the matmuls. rmsnorm
    scaling is deferred through the relu (relu(r*u) = r*relu(u), r>0) so
    the PE never waits on the stats chain. No second collective: each
    core's (h + down) IS the final output for its tokens.
  - Causality is not hardcoded: the mask input is classified host-side
    into skip / plain / mixed 128x512 blocks; mixed tiles are shipped
    as constants (4 distinct tiles for a causal mask).
"""

import os
import sys

try:  # the axon sitecustomize usually provides concourse already
    import concourse.bass  # noqa: F401
except ImportError:  # pragma: no cover
    sys.path.insert(0, "/opt/trn_rl_repo")

from contextlib import ExitStack

import ml_dtypes
import numpy as np

import concourse.bacc as bacc
import concourse.tile as tile
from concourse import mybir
from concourse.bass_utils import run_bass_kernel_spmd
from concourse.masks import make_identity

F32 = mybir.dt.float32
BF16 = mybir.dt.bfloat16
F16 = mybir.dt.float16
N_CORES = 8
P = 128
QW = 512  # q-tile / token-tile width
EPS = 1e-6
AF = mybir.ActivationFunctionType
ALU = mybir.AluOpType
BF16_NP = ml_dtypes.bfloat16


def ts(i, w):
    return slice(i * w, (i + 1) * w)


def _classify_mask(mask, S):
    """mask: [S, S] additive (q, k). Returns (table, tiles).
    table[(kt, j)] = 'skip' | 'plain' | int mask-tile index.
    tiles: list of [128, QW] float32 arrays in scoresT ([k, q]) layout."""
    table = {}
    tiles = []
    keys = {}
    for j in range(S // QW):
        for kt in range(S // P):
            sub = mask[ts(j, QW), ts(kt, P)]  # [q, k]
            if np.all(sub <= -1e8):
                table[(kt, j)] = "skip"
            elif np.all(sub == 0.0):
                table[(kt, j)] = "plain"
            else:
                t = np.ascontiguousarray(sub.T.astype(np.float32))  # [k, q]
                key = t.tobytes()
                if key not in keys:
                    keys[key] = len(tiles)
                    tiles.append(t)
                table[(kt, j)] = keys[key]
    return table, tiles


def build_program(B, S, D, H, HID, mask_table, n_mask):
    HD = 128
    assert D == (D // P) * P and H * HD == D
    HPC = H // N_CORES            # heads per core
    assert HPC * N_CORES == H
    C = D // P                    # contraction chunks over D
    S_TILES = S // QW             # q tiles per batch
    KT = S // P                   # k tiles per batch
    T = B * S                     # total tokens
    OC = HPC * HD // P            # wo input-channel chunks (== HPC)
    G = 4                         # RS chunks
    CT = T // G                   # tokens per RS chunk (1024)
    SL = CT // N_CORES            # tokens per core-slice per chunk (128)
    TPC = G * SL                  # ffn tokens per core (512)
    assert TPC == QW
    HIDC = HID // P               # hidden tiles total (64)
    W1CH = 16                     # w1 stream chunks
    HTPC = HIDC // W1CH           # hid tiles per w1 chunk (4)

    nc = bacc.Bacc(trn_type="TRN2", num_devices=N_CORES)

    xt = nc.dram_tensor("xt", [B, D, S], F32, kind="ExternalInput").ap()
    wq = nc.dram_tensor("wq", [C, P, HPC * HD], BF16, kind="ExternalInput").ap()
    wk = nc.dram_tensor("wk", [C, P, HPC * HD], BF16, kind="ExternalInput").ap()
    wv = nc.dram_tensor("wv", [C, P, HPC * HD], BF16, kind="ExternalInput").ap()
    wo = nc.dram_tensor("wo", [OC, P, D], BF16, kind="ExternalInput").ap()
    w1h = nc.dram_tensor("w1h", [W1CH, P, C, QW], BF16, kind="ExternalInput").ap()
    w2h = nc.dram_tensor("w2h", [C, P, HIDC, P], BF16, kind="ExternalInput").ap()
    mk = None
    if n_mask:
        mk = nc.dram_tensor("mk", [n_mask, P, QW], F32, kind="ExternalInput").ap()

    rs_in = [
        nc.dram_tensor(f"rs_in{g}", [N_CORES, P, C, SL], F16) for g in range(G)
    ]
    rs_out = [nc.dram_tensor(f"rs_out{g}", [P, C, SL], F16) for g in range(G)]
    out = nc.dram_tensor("out", [C, P, QW], F32, kind="ExternalOutput").ap()

    groups = [list(range(N_CORES))]

    with tile.TileContext(nc) as tc, ExitStack() as ctx:
        const = ctx.enter_context(tc.tile_pool(name="const", bufs=1))
        stats = ctx.enter_context(tc.tile_pool(name="stats", bufs=2))
        sqp = ctx.enter_context(tc.tile_pool(name="sq", bufs=2))
        evp = ctx.enter_context(tc.tile_pool(name="ev", bufs=4))
        psum = ctx.enter_context(tc.tile_pool(name="psum", bufs=1, space="PSUM"))

        ones_f32 = const.tile([P, P], F32)
        nc.vector.memset(ones_f32[:], 1.0)
        ones = const.tile([P, P], BF16)
        nc.vector.tensor_copy(ones[:], ones_f32[:])
        eps_p1 = const.tile([P, 1], F32)
        nc.vector.memset(eps_p1[:], EPS)
        ident = const.tile([P, P], F32)
        make_identity(nc, ident[:])

        # ---------------- attention phase ----------------
        with tc.tile_pool(name="wqkv", bufs=1) as wqkvp, \
             tc.tile_pool(name="xa", bufs=4) as xap, \
             tc.tile_pool(name="qkv", bufs=1) as qkvp, \
             tc.tile_pool(name="exp", bufs=2) as expp, \
             tc.tile_pool(name="attn", bufs=1) as attp:
            mtiles = None
            if n_mask:
                mtiles = wqkvp.tile([P, n_mask, QW], F32, tag="mk")
                nc.sync.dma_start(mtiles[:], mk.rearrange("n p q -> p n q"))
            wq_sb = wqkvp.tile([P, C, HPC * HD], BF16, tag="wq")
            nc.sync.dma_start(wq_sb[:], wq.rearrange("c p o -> p c o"))
            wk_sb = wqkvp.tile([P, C, HPC * HD], BF16, tag="wk")
            nc.sync.dma_start(wk_sb[:], wk.rearrange("c p o -> p c o"))
            wv_sb = wqkvp.tile([P, C, HPC * HD], BF16, tag="wv")
            nc.sync.dma_start(wv_sb[:], wv.rearrange("c p o -> p c o"))
            wo_sb = wqkvp.tile([P, OC, D], BF16, tag="wo")
            nc.sync.dma_start(wo_sb[:], wo.rearrange("c p o -> p c o"))

            pending_rs = None

            def emit_rs(g):
                nc.gpsimd.collective_compute(
                    "ReduceScatter",
                    ALU.add,
                    replica_groups=groups,
                    ins=[rs_in[g].ap().opt()],
                    outs=[rs_out[g].ap().opt()],
                )

            for b in range(B):
                # prefetch all 4 x tiles of this batch (bf16 via cast DMA)
                # BEFORE any pending collective blocks the gpsimd queue.
                xbs = []
                for j in range(S_TILES):
                    xb = xap.tile([P, C, QW], BF16, tag="xb")
                    nc.gpsimd.dma_start(
                        xb[:],
                        xt[b].rearrange("(c p) t -> p c t", p=P)[:, :, ts(j, QW)],
                    )
                    xbs.append(xb)
                if pending_rs is not None:
                    emit_rs(pending_rs)
                    pending_rs = None

                kT = qkvp.tile([P, HPC, S], BF16, tag="kT")
                vN = qkvp.tile([P, KT, HPC * HD], BF16, tag="vN")
                for j in range(S_TILES):
                    xb = xbs[j]
                    # rmsnorm stats (replicated over partitions)
                    cs = psum.tile([P, QW], F32, tag="cs", bufs=1)
                    for c in range(C):
                        sq = sqp.tile([P, QW], BF16, tag="sq")
                        nc.scalar.activation(sq[:], xb[:, c, :], AF.Square)
                        nc.tensor.matmul(
                            cs[:], ones[:], sq[:], start=(c == 0), stop=(c == C - 1)
                        )
                    rms = stats.tile([P, QW], F32, tag="rms")
                    nc.scalar.activation(
                        rms[:], cs[:], AF.Sqrt, bias=eps_p1[:], scale=1.0 / D
                    )
                    rinv = stats.tile([P, QW], F32, tag="rinv")
                    nc.vector.reciprocal(rinv[:], rms[:])
                    # q/k projections from RAW x; rinv folded in at eviction
                    qTs = qkvp.tile([P, HPC, QW], BF16, tag="qT", bufs=2)
                    for h in range(HPC):
                        for w_sb, dst in ((wq_sb, qTs), (wk_sb, kT)):
                            pp = psum.tile([P, QW], F32, tag="mm", bufs=3)
                            for c in range(C):
                                nc.tensor.matmul(
                                    pp[:],
                                    w_sb[:, c, ts(h, HD)],
                                    xb[:, c, :],
                                    start=(c == 0),
                                    stop=(c == C - 1),
                                )
                            if dst is qTs:
                                nc.vector.tensor_mul(qTs[:, h, :], pp[:], rinv[:])
                            else:
                                nc.vector.tensor_mul(
                                    kT[:, h, ts(j, QW)], pp[:], rinv[:]
                                )
                    # rinv transposed to token-partition layout for v scaling
                    rcol = stats.tile([P, QW // P], F32, tag="rcol")
                    for sub in range(QW // P):
                        tp = psum.tile([P, P], F32, tag="score", bufs=2)
                        nc.tensor.transpose(tp[:], rinv[:, ts(sub, P)], ident[:])
                        nc.vector.tensor_copy(rcol[:, sub : sub + 1], tp[:, 0:1])
                    # v in natural layout; rinv via per-partition scalar
                    for sub in range(QW // P):
                        pv = psum.tile([P, QW], F32, tag="mm", bufs=3)
                        for c in range(C):
                            nc.tensor.matmul(
                                pv[:, : HPC * HD],
                                xb[:, c, ts(sub, P)],
                                wv_sb[:, c, :],
                                start=(c == 0),
                                stop=(c == C - 1),
                            )
                        nc.vector.tensor_scalar_mul(
                            vN[:, j * (QW // P) + sub, :],
                            pv[:, : HPC * HD],
                            rcol[:, sub : sub + 1],
                        )

                    # -------- attention for q-tile j --------
                    attnT = attp.tile([P, HPC, QW], BF16, tag="attnT", bufs=2)
                    for h in range(HPC):
                        kts = [
                            kt for kt in range(KT) if mask_table[(kt, j)] != "skip"
                        ]
                        pa = psum.tile([P, QW], F32, tag="pv", bufs=1)
                        den = psum.tile([P, QW], F32, tag="stat", bufs=1)
                        n_k = len(kts)
                        exs = [None] * n_k

                        def _den_pv(i):
                            kt = kts[i]
                            nc.tensor.matmul(
                                den[:], ones[:], exs[i][:],
                                start=(i == 0), stop=(i == n_k - 1),
                            )
                            nc.tensor.matmul(
                                pa[:],
                                vN[:, kt, ts(h, HD)],
                                exs[i][:],
                                start=(i == 0),
                                stop=(i == n_k - 1),
                            )

                        for i, kt in enumerate(kts):
                            msc = psum.tile([P, QW], F32, tag="score", bufs=2)
                            nc.tensor.matmul(
                                msc[:],
                                kT[:, h, ts(kt, P)],
                                qTs[:, h, :],
                                start=True,
                                stop=True,
                            )
                            ex = expp.tile([P, QW], BF16, tag="exp")
                            mt = mask_table[(kt, j)]
                            if mt == "plain":
                                nc.scalar.activation(ex[:], msc[:], AF.Exp)
                            else:
                                nc.vector.tensor_add(ex[:], msc[:], mtiles[:, mt, :])
                                nc.scalar.activation(ex[:], ex[:], AF.Exp)
                            exs[i] = ex
                            if i > 0:
                                _den_pv(i - 1)
                        _den_pv(n_k - 1)
                        rec = stats.tile([P, QW], F32, tag="rms")
                        nc.vector.reciprocal(rec[:], den[:])
                        nc.vector.tensor_mul(attnT[:, h, :], pa[:], rec[:])

                    # -------- wo partial + x/8 -> fp16 token-scattered RS in --------
                    g = 2 * b + j // 2
                    r = j % 2  # which half of the chunk this j-tile is
                    for ot in range(C):
                        po = psum.tile([P, QW], F32, tag="mm", bufs=3)
                        for oc in range(OC):
                            nc.tensor.matmul(
                                po[:],
                                wo_sb[:, oc, ts(ot, P)],
                                attnT[:, oc, :],
                                start=(oc == 0),
                                stop=(oc == OC - 1),
                            )
                        ev = evp.tile([P, QW], F16, tag="ev")
                        nc.vector.scalar_tensor_tensor(
                            ev[:], xb[:, ot, :], 1.0 / N_CORES, po[:],
                            op0=ALU.mult, op1=ALU.add,
                        )
                        nc.sync.dma_start(
                            rs_in[g].ap()[4 * r : 4 * r + 4, :, ot, :].rearrange(
                                "s p t -> p s t"
                            ),
                            ev[:],
                        )
                    if j == 1:
                        emit_rs(2 * b)
                    elif j == S_TILES - 1:
                        pending_rs = 2 * b + 1
            assert pending_rs is not None
            emit_rs(pending_rs)

        # ---------------- FFN phase (data-parallel, 512 tokens/core) --------
        with tc.tile_pool(name="hf", bufs=1) as hfp, \
             tc.tile_pool(name="w1p", bufs=2) as w1p, \
             tc.tile_pool(name="w2p", bufs=2) as w2p, \
             tc.tile_pool(name="up", bufs=1) as upp, \
             tc.tile_pool(name="ur", bufs=2) as urp, \
             tc.tile_pool(name="oev", bufs=3) as oevp:
            hf = hfp.tile([P, C, QW], F16, tag="hf")
            for g in range(G):
                nc.sync.dma_start(hf[:, :, ts(g, SL)], rs_out[g].ap())

            # rmsnorm2 stats (consumed only at relu eviction; PE never waits)
            cs = psum.tile([P, QW], F32, tag="cs", bufs=1)
            for c in range(C):
                sq = sqp.tile([P, QW], BF16, tag="sq")
                nc.scalar.activation(sq[:], hf[:, c, :], AF.Square)
                nc.tensor.matmul(
                    cs[:], ones[:], sq[:], start=(c == 0), stop=(c == C - 1)
                )
            rms = stats.tile([P, QW], F32, tag="rms")
            nc.scalar.activation(
                rms[:], cs[:], AF.Sqrt, bias=eps_p1[:], scale=1.0 / D
            )
            r2 = stats.tile([P, QW], F32, tag="rinv")
            nc.vector.reciprocal(r2[:], rms[:])

            up_sb = upp.tile([P, HIDC, QW], BF16, tag="up")
            for ch in range(W1CH):
                w1c = w1p.tile([P, C, QW], BF16, tag="w1")
                nc.sync.dma_start(w1c[:], w1h[ch])
                for hti in range(HTPC):
                    pu = psum.tile([P, QW], F32, tag="mm", bufs=3)
                    for c in range(C):
                        nc.tensor.matmul(
                            pu[:],
                            w1c[:, c, ts(hti, P)],
                            hf[:, c, :],
                            start=(c == 0),
                            stop=(c == C - 1),
                        )
                    ur = urp.tile([P, QW], BF16, tag="ur")
                    nc.scalar.activation(ur[:], pu[:], AF.Relu)
                    nc.vector.tensor_mul(
                        up_sb[:, ch * HTPC + hti, :], ur[:], r2[:]
                    )

            for ot in range(C):
                w2c = w2p.tile([P, HIDC, P], BF16, tag="w2")
                nc.sync.dma_start(w2c[:], w2h[ot])
                pd = psum.tile([P, QW], F32, tag="mm", bufs=3)
                for hc in range(HIDC):
                    nc.tensor.matmul(
                        pd[:],
                        w2c[:, hc, :],
                        up_sb[:, hc, :],
                        start=(hc == 0),
                        stop=(hc == HIDC - 1),
                    )
                oev = oevp.tile([P, QW], F32, tag="oev")
                nc.vector.scalar_tensor_tensor(
                    oev[:], hf[:, ot, :], 1.0, pd[:],
                    op0=ALU.mult, op1=ALU.add,
                )
                nc.sync.dma_start(out[ot], oev[:])

    nc.compile()
    return nc


_CACHE = {}
LAST_RESULT = None


def _get_program(B, S, D, H, HID, mask_table, n_mask, mask_key):
    key = (B, S, D, H, HID, mask_key)
    if key not in _CACHE:
        _CACHE[key] = build_program(B, S, D, H, HID, mask_table, n_mask)
    return _CACHE[key]


def kernel(x, mask, wq, wk, wv, wo, w1, w2, attn_norm_w, ffn_norm_w):
    x = np.asarray(x, dtype=np.float32)
    mask = np.asarray(mask, dtype=np.float32)
    wq, wk, wv, wo = (np.asarray(a, dtype=np.float32) for a in (wq, wk, wv, wo))
    w1, w2 = np.asarray(w1, dtype=np.float32), np.asarray(w2, dtype=np.float32)
    attn_norm_w = np.asarray(attn_norm_w, dtype=np.float32)
    ffn_norm_w = np.asarray(ffn_norm_w, dtype=np.float32)

    B, S, D = x.shape
    H = D // 128  # HD is fixed at 128 (= SBUF partition count)
    HID = w1.shape[0]
    HD = D // H
    HPC = H // N_CORES
    C = D // P
    HIDC = HID // P
    W1CH = 16
    G = 4
    SL = B * S // G // N_CORES

    mask_table, mtiles_np = _classify_mask(
        np.broadcast_to(mask, (1, 1, S, S))[0, 0], S
    )
    mask_key = hash(tuple(sorted((k, str(v)) for k, v in mask_table.items())))
    nc = _get_program(B, S, D, H, HID, mask_table, len(mtiles_np), mask_key)

    # ---- host-side prep ----
    xt = np.ascontiguousarray(x.transpose(0, 2, 1))  # [B, D, S]
    wq_f = (wq * attn_norm_w[None, :]) / np.sqrt(HD)
    wk_f = wk * attn_norm_w[None, :]
    wv_f = wv * attn_norm_w[None, :]
    w1_f = w1 * ffn_norm_w[None, :]

    # full FFN weights, replicated on every core (data-parallel FFN)
    # w1h[ch, p, c, o] = w1_f[hid = ch*512 + o, d = c*128 + p]
    w1host = np.ascontiguousarray(
        w1_f.reshape(W1CH, QW, C, P).transpose(0, 3, 2, 1)
    ).astype(BF16_NP)
    # w2h[ot, p, hc, o] = w2[d_out = ot*128 + o, hid = hc*128 + p]
    w2host = np.ascontiguousarray(
        w2.reshape(C, P, HIDC, P).transpose(0, 3, 2, 1)
    ).astype(BF16_NP)

    in_maps = []
    for c in range(N_CORES):
        hs = slice(c * HPC * HD, (c + 1) * HPC * HD)
        qs = np.ascontiguousarray(wq_f[hs].T).reshape(C, P, HPC * HD).astype(BF16_NP)
        ks = np.ascontiguousarray(wk_f[hs].T).reshape(C, P, HPC * HD).astype(BF16_NP)
        vs = np.ascontiguousarray(wv_f[hs].T).reshape(C, P, HPC * HD).astype(BF16_NP)
        os_ = np.ascontiguousarray(wo[:, hs].T).reshape(HPC, P, D).astype(BF16_NP)
        m = {
            "xt": xt,
            "wq": qs,
            "wk": ks,
            "wv": vs,
            "wo": os_,
            "w1h": w1host,
            "w2h": w2host,
        }
        if len(mtiles_np):
            m["mk"] = np.stack(mtiles_np)
        in_maps.append(m)

    trace = os.environ.get("KTRACE", "0") == "1"
    res = run_bass_kernel_spmd(nc, in_maps, list(range(N_CORES)), trace=trace)
    global LAST_RESULT
    LAST_RESULT = res

    # out[c] is [C, P, QW] f32: column block g*SL+t = token
    # 2048*(g//2) + 512*(2*(g%2) + c//4) + 128*(c%4) + t
    full = np.empty((B * S, D), dtype=np.float32)
    for core in range(N_CORES):
        o = res.results[core]["out"].reshape(D, QW)  # [C*P, QW] = [D, 512]
        for g in range(G):
            tok0 = S * (g // 2) + QW * (2 * (g % 2) + core // 4) + SL * (core % 4)
            full[tok0 : tok0 + SL, :] = o[:, ts(g, SL)].T
    return np.ascontiguousarray(full.reshape(B, S, D))


# revision 6
# speedup vs baseline: 1.5906x; 1.0076x over previous
"""Trainium2 Bass kernel for a dense transformer decoder block.

Strategy (8 NeuronCores):
  - Attention tensor-parallel over heads (2 heads/core); activations in
    transposed layout [D, tokens]; all matmuls bf16 with fp32 PSUM.
  - wo partials (+ x/8 residual fold) are written in fp16 into
    token-scattered ReduceScatter buffers: 5 chunked RS collectives
    issued mid-attention so they overlap compute (the last chunk is a
    single q-tile to minimize the exposed tail). The RS output hands
    each core h for its own 512 tokens.
  - FFN is data-parallel: each core runs the FULL FFN on its 512 tokens,
    streaming the full w1/w2 (bf16) from HBM under the matmuls. rmsnorm
    scaling is deferred through relu and the (linear) down-projection
    (relu(r*u) = r*relu(u), r>0), applied once on the 16 down outputs,
    so the PE never waits on the stats chain. No second collective:
    each core's (h + down) IS the final output for its tokens.
  - Engine balance: squares on DVE, exp/reciprocal on ACT (table swaps
    minimized), rmsnorm rsqrt on DVE off the critical path; stats are
    computed one q-tile ahead so PE runs dense.
  - Causality is not hardcoded: the mask input is classified host-side
    into skip / plain / mixed 128x512 blocks; mixed tiles are shipped
    as constants (4 distinct tiles for a causal mask).
"""

import os
import sys

try:  # the axon sitecustomize usually provides concourse already
    import concourse.bass  # noqa: F401
except ImportError:  # pragma: no cover
    sys.path.insert(0, "/opt/trn_rl_repo")

from contextlib import ExitStack

import ml_dtypes
import numpy as np

import concourse.bacc as bacc
import concourse.tile as tile
from concourse import mybir
from concourse.bass_utils import run_bass_kernel_spmd
from concourse.masks import make_identity

F32 = mybir.dt.float32
BF16 = mybir.dt.bfloat16
F16 = mybir.dt.float16
N_CORES = 8
P = 128
QW = 512  # q-tile / token-tile width
EPS = 1e-6
AF = mybir.ActivationFunctionType
ALU = mybir.AluOpType
BF16_NP = ml_dtypes.bfloat16

# RS chunks: (batch, q-tiles covered). Last two are single-tile so the
# exposed tail collective is small.
CHUNKS = [(0, (0, 1)), (0, (2, 3)), (1, (0, 1)), (1, (2,)), (1, (3,))]


def ts(i, w):
    return slice(i * w, (i + 1) * w)


def _classify_mask(mask, S):
    """mask: [S, S] additive (q, k). Returns (table, tiles).
    table[(kt, j)] = 'skip' | 'plain' | int mask-tile index.
    tiles: list of [128, QW] float32 arrays in scoresT ([k, q]) layout."""
    table = {}
    tiles = []
    keys = {}
    for j in range(S // QW):
        for kt in range(S // P):
            sub = mask[ts(j, QW), ts(kt, P)]  # [q, k]
            if np.all(sub <= -1e8):
                table[(kt, j)] = "skip"
            elif np.all(sub == 0.0):
                table[(kt, j)] = "plain"
            else:
                t = np.ascontiguousarray(sub.T.astype(np.float32))  # [k, q]
                key = t.tobytes()
                if key not in keys:
                    keys[key] = len(tiles)
                    tiles.append(t)
                table[(kt, j)] = keys[key]
    return table, tiles


def build_program(B, S, D, H, HID, mask_table, n_mask):
    HD = 128
    assert D == (D // P) * P and H * HD == D
    HPC = H // N_CORES            # heads per core
    assert HPC * N_CORES == H
    C = D // P                    # contraction chunks over D
    S_TILES = S // QW             # q tiles per batch
    KT = S // P                   # k tiles per batch
    OC = HPC * HD // P            # wo input-channel chunks (== HPC)
    G = len(CHUNKS)
    HIDC = HID // P               # hidden tiles total (64)
    W1CH = 16                     # w1 stream chunks
    HTPC = HIDC // W1CH           # hid tiles per w1 chunk (4)

    chunk_of = {}                 # (b, j) -> (g, r)
    slw = []                      # slice width per chunk
    col0 = []                     # ffn column start per chunk
    acc = 0
    for g, (b, js) in enumerate(CHUNKS):
        for r, j in enumerate(js):
            chunk_of[(b, j)] = (g, r)
        slw.append(QW * len(js) // N_CORES)
        col0.append(acc)
        acc += slw[-1]
    assert acc == QW

    nc = bacc.Bacc(trn_type="TRN2", num_devices=N_CORES)

    xt = nc.dram_tensor("xt", [B, D, S], F32, kind="ExternalInput").ap()
    wq = nc.dram_tensor("wq", [C, P, HPC * HD], BF16, kind="ExternalInput").ap()
    wk = nc.dram_tensor("wk", [C, P, HPC * HD], BF16, kind="ExternalInput").ap()
    wv = nc.dram_tensor("wv", [C, P, HPC * HD], BF16, kind="ExternalInput").ap()
    wo = nc.dram_tensor("wo", [OC, P, D], BF16, kind="ExternalInput").ap()
    w1h = nc.dram_tensor("w1h", [W1CH, P, C, QW], BF16, kind="ExternalInput").ap()
    w2h = nc.dram_tensor("w2h", [C, P, HIDC, P], BF16, kind="ExternalInput").ap()
    mk = None
    if n_mask:
        mk = nc.dram_tensor("mk", [n_mask, P, QW], F32, kind="ExternalInput").ap()

    rs_in = [
        nc.dram_tensor(f"rs_in{g}", [N_CORES, P, C, slw[g]], F16)
        for g in range(G)
    ]
    rs_out = [
        nc.dram_tensor(f"rs_out{g}", [P, C, slw[g]], F16) for g in range(G)
    ]
    out = nc.dram_tensor("out", [C, P, QW], F32, kind="ExternalOutput").ap()

    groups = [list(range(N_CORES))]

    with tile.TileContext(nc) as tc, ExitStack() as ctx:
        const = ctx.enter_context(tc.tile_pool(name="const", bufs=1))
        stats = ctx.enter_context(tc.tile_pool(name="stats", bufs=2))
        sqp = ctx.enter_context(tc.tile_pool(name="sq", bufs=2))
        evp = ctx.enter_context(tc.tile_pool(name="ev", bufs=4))
        psum = ctx.enter_context(tc.tile_pool(name="psum", bufs=1, space="PSUM"))

        ones_f32 = const.tile([P, P], F32)
        nc.vector.memset(ones_f32[:], 1.0)
        ones = const.tile([P, P], BF16)
        nc.vector.tensor_copy(ones[:], ones_f32[:])
        eps_p1 = const.tile([P, 1], F32)
        nc.vector.memset(eps_p1[:], EPS)
        ident = const.tile([P, P], F32)
        make_identity(nc, ident[:])

        # ---------------- attention phase ----------------
        with tc.tile_pool(name="wqkv", bufs=1) as wqkvp, \
             tc.tile_pool(name="xa", bufs=4) as xap, \
             tc.tile_pool(name="qkv", bufs=1) as qkvp, \
             tc.tile_pool(name="exp", bufs=2) as expp, \
             tc.tile_pool(name="attn", bufs=1) as attp:
            mtiles = None
            if n_mask:
                mtiles = wqkvp.tile([P, n_mask, QW], F32, tag="mk")
                nc.sync.dma_start(mtiles[:], mk.rearrange("n p q -> p n q"))
            wq_sb = wqkvp.tile([P, C, HPC * HD], BF16, tag="wq")
            nc.sync.dma_start(wq_sb[:], wq.rearrange("c p o -> p c o"))
            wk_sb = wqkvp.tile([P, C, HPC * HD], BF16, tag="wk")
            nc.sync.dma_start(wk_sb[:], wk.rearrange("c p o -> p c o"))
            wv_sb = wqkvp.tile([P, C, HPC * HD], BF16, tag="wv")
            nc.sync.dma_start(wv_sb[:], wv.rearrange("c p o -> p c o"))
            wo_sb = wqkvp.tile([P, OC, D], BF16, tag="wo")
            nc.sync.dma_start(wo_sb[:], wo.rearrange("c p o -> p c o"))

            def emit_rs(g):
                nc.gpsimd.collective_compute(
                    "ReduceScatter",
                    ALU.add,
                    replica_groups=groups,
                    ins=[rs_in[g].ap().opt()],
                    outs=[rs_out[g].ap().opt()],
                )

            def stats_for(xb):
                """sq on DVE, column sums via PE, rinv = rsqrt on DVE
                (off the PE critical path)."""
                cs = psum.tile([P, QW], F32, tag="cs", bufs=1)
                for c in range(C):
                    sq = sqp.tile([P, QW], BF16, tag="sq")
                    nc.vector.tensor_mul(sq[:], xb[:, c, :], xb[:, c, :])
                    nc.tensor.matmul(
                        cs[:], ones[:], sq[:], start=(c == 0), stop=(c == C - 1)
                    )
                rms = stats.tile([P, QW], F32, tag="rms")
                nc.scalar.activation(
                    rms[:], cs[:], AF.Sqrt, bias=eps_p1[:], scale=1.0 / D
                )
                rinv = stats.tile([P, QW], F32, tag="rinv")
                nc.vector.reciprocal_approx_fast(rinv[:], rms[:])
                return rinv

            pending_rs = None
            for b in range(B):
                # prefetch all 4 x tiles of this batch (bf16 via cast DMA)
                # BEFORE any pending collective blocks the gpsimd queue.
                xbs = []
                for j in range(S_TILES):
                    xb = xap.tile([P, C, QW], BF16, tag="xb")
                    nc.gpsimd.dma_start(
                        xb[:],
                        xt[b].rearrange("(c p) t -> p c t", p=P)[:, :, ts(j, QW)],
                    )
                    xbs.append(xb)
                if pending_rs is not None:
                    emit_rs(pending_rs)
                    pending_rs = None

                rinv_next = stats_for(xbs[0])

                kT = qkvp.tile([P, HPC, S], BF16, tag="kT")
                vN = qkvp.tile([P, KT, HPC * HD], BF16, tag="vN")
                for j in range(S_TILES):
                    xb = xbs[j]
                    rinv = rinv_next
                    # q/k projections from RAW x; rinv folded in at eviction
                    qTs = qkvp.tile([P, HPC, QW], BF16, tag="qT", bufs=2)
                    for h in range(HPC):
                        for w_sb, dst in ((wq_sb, qTs), (wk_sb, kT)):
                            pp = psum.tile([P, QW], F32, tag="mm", bufs=3)
                            for c in range(C):
                                nc.tensor.matmul(
                                    pp[:],
                                    w_sb[:, c, ts(h, HD)],
                                    xb[:, c, :],
                                    start=(c == 0),
                                    stop=(c == C - 1),
                                )
                            if dst is qTs:
                                nc.vector.tensor_mul(qTs[:, h, :], pp[:], rinv[:])
                            else:
                                nc.vector.tensor_mul(
                                    kT[:, h, ts(j, QW)], pp[:], rinv[:]
                                )
                    # rinv transposed to token-partition layout for v scaling
                    rcol = stats.tile([P, QW // P], F32, tag="rcol")
                    for sub in range(QW // P):
                        tp = psum.tile([P, P], F32, tag="cs", bufs=1)
                        nc.tensor.transpose(tp[:], rinv[:, ts(sub, P)], ident[:])
                        nc.vector.tensor_copy(rcol[:, sub : sub + 1], tp[:, 0:1])
                    # v in natural layout; rinv via per-partition scalar
                    for sub in range(QW // P):
                        pv = psum.tile([P, QW], F32, tag="mm", bufs=3)
                        for c in range(C):
                            nc.tensor.matmul(
                                pv[:, : HPC * HD],
                                xb[:, c, ts(sub, P)],
                                wv_sb[:, c, :],
                                start=(c == 0),
                                stop=(c == C - 1),
                            )
                        nc.vector.tensor_scalar_mul(
                            vN[:, j * (QW // P) + sub, :],
                            pv[:, : HPC * HD],
                            rcol[:, sub : sub + 1],
                        )

                    # stats for the NEXT q-tile, emitted here so the PE and
                    # DVE work overlaps attention below (no j-boundary stall)
                    if j + 1 < S_TILES:
                        rinv_next = stats_for(xbs[j + 1])

                    # -------- attention for q-tile j --------
                    attnT = attp.tile([P, HPC, QW], BF16, tag="attnT", bufs=2)
                    for h in range(HPC):
                        kts = [
                            kt for kt in range(KT) if mask_table[(kt, j)] != "skip"
                        ]
                        pa = psum.tile([P, QW], F32, tag="pv", bufs=1)
                        den = psum.tile([P, QW], F32, tag="stat", bufs=1)
                        n_k = len(kts)
                        exs = [None] * n_k

                        def _den_pv(i):
                            kt = kts[i]
                            nc.tensor.matmul(
                                den[:], ones[:], exs[i][:],
                                start=(i == 0), stop=(i == n_k - 1),
                            )
                            nc.tensor.matmul(
                                pa[:],
                                vN[:, kt, ts(h, HD)],
                                exs[i][:],
                                start=(i == 0),
                                stop=(i == n_k - 1),
                            )

                        for i, kt in enumerate(kts):
                            msc = psum.tile([P, QW], F32, tag="score", bufs=2)
                            nc.tensor.matmul(
                                msc[:],
                                kT[:, h, ts(kt, P)],
                                qTs[:, h, :],
                                start=True,
                                stop=True,
                            )
                            ex = expp.tile([P, QW], BF16, tag="exp")
                            mt = mask_table[(kt, j)]
                            if mt == "plain":
                                nc.scalar.activation(ex[:], msc[:], AF.Exp)
                            else:
                                nc.vector.tensor_add(ex[:], msc[:], mtiles[:, mt, :])
                                nc.scalar.activation(ex[:], ex[:], AF.Exp)
                            exs[i] = ex
                            if i > 0:
                                _den_pv(i - 1)
                        _den_pv(n_k - 1)
                        rec = stats.tile([P, QW], F32, tag="rec")
                        nc.vector.reciprocal_approx_fast(rec[:], den[:])
                        nc.vector.tensor_mul(attnT[:, h, :], pa[:], rec[:])

                    # ---- wo partial + x/8 -> fp16 token-scattered RS in ----
                    g, r = chunk_of[(b, j)]
                    nsl = N_CORES // len(CHUNKS[g][1])  # slices from this j
                    for ot in range(C):
                        po = psum.tile([P, QW], F32, tag="mm", bufs=3)
                        for oc in range(OC):
                            nc.tensor.matmul(
                                po[:],
                                wo_sb[:, oc, ts(ot, P)],
                                attnT[:, oc, :],
                                start=(oc == 0),
                                stop=(oc == OC - 1),
                            )
                        ev = evp.tile([P, QW], F16, tag="ev")
                        nc.vector.scalar_tensor_tensor(
                            ev[:], xb[:, ot, :], 1.0 / N_CORES, po[:],
                            op0=ALU.mult, op1=ALU.add,
                        )
                        nc.sync.dma_start(
                            rs_in[g].ap()[
                                nsl * r : nsl * (r + 1), :, ot, :
                            ].rearrange("s p t -> p s t"),
                            ev[:],
                        )
                    if (b, j) == (CHUNKS[g][0], CHUNKS[g][1][-1]):
                        # chunk complete
                        if g == 1:
                            pending_rs = g  # emit after b1's x prefetch
                        else:
                            emit_rs(g)

        # ---------------- FFN phase (data-parallel, 512 tokens/core) --------
        with tc.tile_pool(name="hf", bufs=1) as hfp, \
             tc.tile_pool(name="w1p", bufs=2) as w1p, \
             tc.tile_pool(name="w2p", bufs=2) as w2p, \
             tc.tile_pool(name="up", bufs=1) as upp, \
             tc.tile_pool(name="oev", bufs=3) as oevp:
            # prefetch first w1 chunk BEFORE the RS-gated h loads so the
            # sync queue doesn't stall the weight stream behind RS-4.
            w1cs = [None] * W1CH
            w1c0 = w1p.tile([P, C, QW], BF16, tag="w1", name="w1c0")
            w1cs[0] = w1c0
            nc.sync.dma_start(w1cs[0][:], w1h[0])

            hf = hfp.tile([P, C, QW], F16, tag="hf")
            for g in range(G):
                nc.sync.dma_start(hf[:, :, col0[g] : col0[g] + slw[g]], rs_out[g].ap())

            # rmsnorm2 stats (consumed only at the down outputs; the PE
            # never waits: relu/down are scale-deferred)
            cs = psum.tile([P, QW], F32, tag="cs", bufs=1)
            for c in range(C):
                sq = sqp.tile([P, QW], BF16, tag="sq")
                nc.vector.tensor_mul(sq[:], hf[:, c, :], hf[:, c, :])
                nc.tensor.matmul(
                    cs[:], ones[:], sq[:], start=(c == 0), stop=(c == C - 1)
                )
            rms2 = stats.tile([P, QW], F32, tag="rms")
            nc.scalar.activation(
                rms2[:], cs[:], AF.Sqrt, bias=eps_p1[:], scale=1.0 / D
            )
            r2 = stats.tile([P, QW], F32, tag="rinv")
            nc.vector.reciprocal_approx_fast(r2[:], rms2[:])

            up_sb = upp.tile([P, HIDC, QW], BF16, tag="up")
            for ch in range(W1CH):
                if w1cs[ch] is None:
                    w1cn = w1p.tile([P, C, QW], BF16, tag="w1", name=f"w1c{ch}")
                    w1cs[ch] = w1cn
                    nc.sync.dma_start(w1cs[ch][:], w1h[ch])
                w1c = w1cs[ch]
                for hti in range(HTPC):
                    pu = psum.tile([P, QW], F32, tag="mm", bufs=3)
                    for c in range(C):
                        nc.tensor.matmul(
                            pu[:],
                            w1c[:, c, ts(hti, P)],
                            hf[:, c, :],
                            start=(c == 0),
                            stop=(c == C - 1),
                        )
                    nc.scalar.activation(
                        up_sb[:, ch * HTPC + hti, :], pu[:], AF.Relu
                    )

            for ot in range(C):
                w2c = w2p.tile([P, HIDC, P], BF16, tag="w2")
                nc.sync.dma_start(w2c[:], w2h[ot])
                pd = psum.tile([P, QW], F32, tag="mm", bufs=3)
                for hc in range(HIDC):
                    nc.tensor.matmul(
                        pd[:],
                        w2c[:, hc, :],
                        up_sb[:, hc, :],
                        start=(hc == 0),
                        stop=(hc == HIDC - 1),
                    )
                dn = oevp.tile([P, QW], F32, tag="dn")
                nc.vector.tensor_mul(dn[:], pd[:], r2[:])
                oev = oevp.tile([P, QW], F32, tag="oev")
                nc.vector.tensor_add(oev[:], hf[:, ot, :], dn[:])
                nc.sync.dma_start(out[ot], oev[:])

    nc.compile()
    return nc


_CACHE = {}
LAST_RESULT = None


def _get_program(B, S, D, H, HID, mask_table, n_mask, mask_key):
    key = (B, S, D, H, HID, mask_key)
    if key not in _CACHE:
        _CACHE[key] = build_program(B, S, D, H, HID, mask_table, n_mask)
    return _CACHE[key]


def kernel(x, mask, wq, wk, wv, wo, w1, w2, attn_norm_w, ffn_norm_w):
    x = np.asarray(x, dtype=np.float32)
    mask = np.asarray(mask, dtype=np.float32)
    wq, wk, wv, wo = (np.asarray(a, dtype=np.float32) for a in (wq, wk, wv, wo))
    w1, w2 = np.asarray(w1, dtype=np.float32), np.asarray(w2, dtype=np.float32)
    attn_norm_w = np.asarray(attn_norm_w, dtype=np.float32)
    ffn_norm_w = np.asarray(ffn_norm_w, dtype=np.float32)

    B, S, D = x.shape
    H = D // 128  # HD is fixed at 128 (= SBUF partition count)
    HID = w1.shape[0]
    HD = D // H
    HPC = H // N_CORES
    C = D // P
    HIDC = HID // P
    W1CH = 16

    mask_table, mtiles_np = _classify_mask(
        np.broadcast_to(mask, (1, 1, S, S))[0, 0], S
    )
    mask_key = hash(tuple(sorted((k, str(v)) for k, v in mask_table.items())))
    nc = _get_program(B, S, D, H, HID, mask_table, len(mtiles_np), mask_key)

    # ---- host-side prep ----
    xt = np.ascontiguousarray(x.transpose(0, 2, 1))  # [B, D, S]
    wq_f = (wq * attn_norm_w[None, :]) / np.sqrt(HD)
    wk_f = wk * attn_norm_w[None, :]
    wv_f = wv * attn_norm_w[None, :]
    w1_f = w1 * ffn_norm_w[None, :]

    # full FFN weights, replicated on every core (data-parallel FFN)
    # w1h[ch, p, c, o] = w1_f[hid = ch*512 + o, d = c*128 + p]
    w1host = np.ascontiguousarray(
        w1_f.reshape(W1CH, QW, C, P).transpose(0, 3, 2, 1)
    ).astype(BF16_NP)
    # w2h[ot, p, hc, o] = w2[d_out = ot*128 + o, hid = hc*128 + p]
    w2host = np.ascontiguousarray(
        w2.reshape(C, P, HIDC, P).transpose(0, 3, 2, 1)
    ).astype(BF16_NP)

    in_maps = []
    for c in range(N_CORES):
        hs = slice(c * HPC * HD, (c + 1) * HPC * HD)
        qs = np.ascontiguousarray(wq_f[hs].T).reshape(C, P, HPC * HD).astype(BF16_NP)
        ks = np.ascontiguousarray(wk_f[hs].T).reshape(C, P, HPC * HD).astype(BF16_NP)
        vs = np.ascontiguousarray(wv_f[hs].T).reshape(C, P, HPC * HD).astype(BF16_NP)
        os_ = np.ascontiguousarray(wo[:, hs].T).reshape(HPC, P, D).astype(BF16_NP)
        m = {
            "xt": xt,
            "wq": qs,
            "wk": ks,
            "wv": vs,
            "wo": os_,
            "w1h": w1host,
            "w2h": w2host,
        }
        if len(mtiles_np):
            m["mk"] = np.stack(mtiles_np)
        in_maps.append(m)

    trace = os.environ.get("KTRACE", "0") == "1"
    res = run_bass_kernel_spmd(nc, in_maps, list(range(N_CORES)), trace=trace)
    global LAST_RESULT
    LAST_RESULT = res

    # out[c] is [C, P, QW] f32 = [D, 512]; FFN column ranges map back to
    # tokens per the chunk table.
    full = np.empty((B * S, D), dtype=np.float32)
    col = 0
    starts = []
    for b, js in CHUNKS:
        starts.append(col)
        col += QW * len(js) // N_CORES
    for core in range(N_CORES):
        o = res.results[core]["out"].reshape(D, QW)
        for g, (b, js) in enumerate(CHUNKS):
            sl = QW * len(js) // N_CORES
            if len(js) == 2:
                j = js[core // 4]
                tok0 = S * b + QW * j + sl * (core % 4)
            else:
                j = js[0]
                tok0 = S * b + QW * j + sl * core
            full[tok0 : tok0 + sl, :] = o[:, starts[g] : starts[g] + sl].T
    return np.ascontiguousarray(full.reshape(B, S, D))


# revision 9
# speedup vs baseline: 1.6319x; 1.0260x over previous
"""Trainium2 Bass kernel for a dense transformer decoder block.

Strategy (8 NeuronCores):
  - Attention tensor-parallel over heads (2 heads/core); activations in
    transposed layout [D, tokens]; all matmuls bf16 with fp32 PSUM.
  - wo partials (+ x/8 residual fold) are written in fp16 into
    token-scattered ReduceScatter buffers: 5 chunked RS collectives
    issued mid-attention so they overlap compute (the last chunk is a
    single q-tile to minimize the exposed tail). The RS output hands
    each core h for its own 512 tokens.
  - FFN is data-parallel: each core runs the FULL FFN on its 512 tokens,
    streaming the full w1/w2 (bf16) from HBM under the matmuls. rmsnorm
    scaling is deferred through relu and the (linear) down-projection
    (relu(r*u) = r*relu(u), r>0), applied once on the 16 down outputs,
    so the PE never waits on the stats chain. No second collective:
    each core's (h + down) IS the final output for its tokens.
  - Engine balance: squares on DVE, exp/reciprocal on ACT (table swaps
    minimized), rmsnorm rsqrt on DVE off the critical path; stats are
    computed one q-tile ahead so PE runs dense.
  - Causality is not hardcoded: the mask input is classified host-side
    into skip / plain / mixed 128x512 blocks; mixed tiles are shipped
    as constants (4 distinct tiles for a causal mask).
"""

import os
import sys

try:  # the axon sitecustomize usually provides concourse already
    import concourse.bass  # noqa: F401
except ImportError:  # pragma: no cover
    sys.path.insert(0, "/opt/trn_rl_repo")

from contextlib import ExitStack

import ml_dtypes
import numpy as np

import concourse.bacc as bacc
import concourse.tile as tile
from concourse import mybir
from concourse.bass_utils import run_bass_kernel_spmd
from concourse.masks import make_identity

F32 = mybir.dt.float32
BF16 = mybir.dt.bfloat16
F16 = mybir.dt.float16
N_CORES = 8
P = 128
QW = 512  # q-tile / token-tile width
EPS = 1e-6
AF = mybir.ActivationFunctionType
ALU = mybir.AluOpType
BF16_NP = ml_dtypes.bfloat16

# RS chunks: (batch, q-tiles covered). Last two are single-tile so the
# exposed tail collective is small.
CHUNKS = [(0, (0, 1)), (0, (2, 3)), (1, (0, 1)), (1, (2,)), (1, (3,))]


def ts(i, w):
    return slice(i * w, (i + 1) * w)


def _classify_mask(mask, S):
    """mask: [S, S] additive (q, k). Returns (table, tiles).
    table[(kt, j)] = 'skip' | 'plain' | int mask-tile index.
    tiles: list of [128, QW] float32 arrays in scoresT ([k, q]) layout."""
    table = {}
    tiles = []
    keys = {}
    for j in range(S // QW):
        for kt in range(S // P):
            sub = mask[ts(j, QW), ts(kt, P)]  # [q, k]
            if np.all(sub <= -1e8):
                table[(kt, j)] = "skip"
            elif np.all(sub == 0.0):
                table[(kt, j)] = "plain"
            else:
                t = np.ascontiguousarray(sub.T.astype(np.float32))  # [k, q]
                key = t.tobytes()
                if key not in keys:
                    keys[key] = len(tiles)
                    tiles.append(t)
                table[(kt, j)] = keys[key]
    return table, tiles


def build_program(B, S, D, H, HID, mask_table, n_mask):
    HD = 128
    assert D == (D // P) * P and H * HD == D
    HPC = H // N_CORES            # heads per core
    assert HPC * N_CORES == H
    C = D // P                    # contraction chunks over D
    S_TILES = S // QW             # q tiles per batch
    KT = S // P                   # k tiles per batch
    OC = HPC * HD // P            # wo input-channel chunks (== HPC)
    G = len(CHUNKS)
    HIDC = HID // P               # hidden tiles total (64)
    W1CH = 16                     # w1 stream chunks
    HTPC = HIDC // W1CH           # hid tiles per w1 chunk (4)

    chunk_of = {}                 # (b, j) -> (g, r)
    slw = []                      # slice width per chunk
    col0 = []                     # ffn column start per chunk
    acc = 0
    for g, (b, js) in enumerate(CHUNKS):
        for r, j in enumerate(js):
            chunk_of[(b, j)] = (g, r)
        slw.append(QW * len(js) // N_CORES)
        col0.append(acc)
        acc += slw[-1]
    assert acc == QW

    nc = bacc.Bacc(trn_type="TRN2", num_devices=N_CORES)

    xt = nc.dram_tensor("xt", [B, D, S], F32, kind="ExternalInput").ap()
    wq = nc.dram_tensor("wq", [C, P, HPC * HD], BF16, kind="ExternalInput").ap()
    wk = nc.dram_tensor("wk", [C, P, HPC * HD], BF16, kind="ExternalInput").ap()
    wv = nc.dram_tensor("wv", [C, P, HPC * HD], BF16, kind="ExternalInput").ap()
    wo = nc.dram_tensor("wo", [OC, P, D], BF16, kind="ExternalInput").ap()
    w1h = nc.dram_tensor("w1h", [W1CH, P, C, QW], BF16, kind="ExternalInput").ap()
    w2h = nc.dram_tensor("w2h", [C, P, HIDC, P], BF16, kind="ExternalInput").ap()
    mk = None
    if n_mask:
        mk = nc.dram_tensor("mk", [n_mask, P, QW], BF16, kind="ExternalInput").ap()

    rs_in = [
        nc.dram_tensor(f"rs_in{g}", [N_CORES, P, C, slw[g]], F16)
        for g in range(G)
    ]
    rs_out = [
        nc.dram_tensor(f"rs_out{g}", [P, C, slw[g]], F16) for g in range(G)
    ]
    out = nc.dram_tensor("out", [C, P, QW], F32, kind="ExternalOutput").ap()

    groups = [list(range(N_CORES))]

    with tile.TileContext(nc) as tc, ExitStack() as ctx:
        const = ctx.enter_context(tc.tile_pool(name="const", bufs=1))
        stats = ctx.enter_context(tc.tile_pool(name="stats", bufs=2))
        sqp = ctx.enter_context(tc.tile_pool(name="sq", bufs=2))
        evp = ctx.enter_context(tc.tile_pool(name="ev", bufs=4))
        psum = ctx.enter_context(tc.tile_pool(name="psum", bufs=1, space="PSUM"))

        ones_f32 = const.tile([P, P], F32)
        nc.vector.memset(ones_f32[:], 1.0)
        ones = const.tile([P, P], BF16)
        nc.vector.tensor_copy(ones[:], ones_f32[:])
        eps_p1 = const.tile([P, 1], F32)
        nc.vector.memset(eps_p1[:], EPS)
        ident = const.tile([P, P], F32)
        make_identity(nc, ident[:])
        ident_b = const.tile([P, P], BF16)
        nc.vector.tensor_copy(ident_b[:], ident[:])

        # ---------------- attention phase ----------------
        with tc.tile_pool(name="wqkv", bufs=1) as wqkvp, \
             tc.tile_pool(name="xa", bufs=4) as xap, \
             tc.tile_pool(name="qkv", bufs=1) as qkvp, \
             tc.tile_pool(name="exp", bufs=3) as expp, \
             tc.tile_pool(name="attn", bufs=1) as attp:
            mtiles = None
            if n_mask:
                mtiles = wqkvp.tile([P, n_mask, QW], BF16, tag="mk")
                nc.sync.dma_start(mtiles[:], mk.rearrange("n p q -> p n q"))
            wq_sb = wqkvp.tile([P, C, HPC * HD], BF16, tag="wq")
            nc.sync.dma_start(wq_sb[:], wq.rearrange("c p o -> p c o"))
            wk_sb = wqkvp.tile([P, C, HPC * HD], BF16, tag="wk")
            nc.sync.dma_start(wk_sb[:], wk.rearrange("c p o -> p c o"))
            wv_sb = wqkvp.tile([P, C, HPC * HD], BF16, tag="wv")
            nc.sync.dma_start(wv_sb[:], wv.rearrange("c p o -> p c o"))
            wo_sb = wqkvp.tile([P, OC, D], BF16, tag="wo")
            nc.sync.dma_start(wo_sb[:], wo.rearrange("c p o -> p c o"))

            def emit_rs(g):
                nc.gpsimd.collective_compute(
                    "ReduceScatter",
                    ALU.add,
                    replica_groups=groups,
                    ins=[rs_in[g].ap().opt()],
                    outs=[rs_out[g].ap().opt()],
                )

            def stats_mm(xb):
                """sq on DVE, column sums via PE."""
                cs = psum.tile([P, QW], F32, tag="mm", bufs=3)
                for c in range(C):
                    sq = sqp.tile([P, QW], BF16, tag="sq")
                    nc.vector.tensor_mul(sq[:], xb[:, c, :], xb[:, c, :])
                    nc.tensor.matmul(
                        cs[:], ones[:], sq[:], start=(c == 0), stop=(c == C - 1)
                    )
                return cs

            def stats_fin(cs):
                """sqrt on ACT (emitted outside the exp burst) + fast recip."""
                rms = stats.tile([P, QW], F32, tag="rms")
                nc.scalar.activation(
                    rms[:], cs[:], AF.Sqrt, bias=eps_p1[:], scale=1.0 / D
                )
                rinv = stats.tile([P, QW], F32, tag="rinv")
                nc.vector.reciprocal_approx_fast(rinv[:], rms[:])
                return rinv

            pending_rs = None
            for b in range(B):
                # prefetch all 4 x tiles of this batch (bf16 via cast DMA)
                # BEFORE any pending collective blocks the gpsimd queue.
                xbs = []
                for j in range(S_TILES):
                    xb = xap.tile([P, C, QW], BF16, tag="xb")
                    nc.gpsimd.dma_start(
                        xb[:],
                        xt[b].rearrange("(c p) t -> p c t", p=P)[:, :, ts(j, QW)],
                    )
                    xbs.append(xb)
                if pending_rs is not None:
                    emit_rs(pending_rs)
                    pending_rs = None

                rinv_next = stats_fin(stats_mm(xbs[0]))

                kT = qkvp.tile([P, HPC, S], BF16, tag="kT")
                vN = qkvp.tile([P, KT, HPC * HD], BF16, tag="vN")
                for j in range(S_TILES):
                    xb = xbs[j]
                    rinv = rinv_next
                    # q/k projections from RAW x; rinv folded in at eviction
                    qTs = qkvp.tile([P, HPC, QW], BF16, tag="qT", bufs=2)
                    for h in range(HPC):
                        for w_sb, dst in ((wq_sb, qTs), (wk_sb, kT)):
                            pp = psum.tile([P, QW], F32, tag="mm", bufs=3)
                            for c in range(C):
                                nc.tensor.matmul(
                                    pp[:],
                                    w_sb[:, c, ts(h, HD)],
                                    xb[:, c, :],
                                    start=(c == 0),
                                    stop=(c == C - 1),
                                )
                            if dst is qTs:
                                nc.vector.tensor_mul(qTs[:, h, :], pp[:], rinv[:])
                            else:
                                nc.vector.tensor_mul(
                                    kT[:, h, ts(j, QW)], pp[:], rinv[:]
                                )
                    # rinv transposed to token-partition layout for v scaling
                    rcol = stats.tile([P, QW // P], F32, tag="rcol")
                    for sub in range(QW // P):
                        tp = psum.tile([P, P], F32, tag="mm", bufs=3)
                        nc.tensor.transpose(tp[:], rinv[:, ts(sub, P)], ident[:])
                        nc.vector.tensor_copy(rcol[:, sub : sub + 1], tp[:, 0:1])
                    # v in natural layout; rinv via per-partition scalar
                    for sub in range(QW // P):
                        pv = psum.tile([P, QW], F32, tag="mm", bufs=3)
                        for c in range(C):
                            nc.tensor.matmul(
                                pv[:, : HPC * HD],
                                xb[:, c, ts(sub, P)],
                                wv_sb[:, c, :],
                                start=(c == 0),
                                stop=(c == C - 1),
                            )
                        nc.vector.tensor_scalar_mul(
                            vN[:, j * (QW // P) + sub, :],
                            pv[:, : HPC * HD],
                            rcol[:, sub : sub + 1],
                        )

                    # stats matmuls for the NEXT q-tile, emitted here so the
                    # PE/DVE work overlaps attention below; the ACT sqrt is
                    # emitted after the exp burst (table-swap off the chain)
                    cs_next = stats_mm(xbs[j + 1]) if j + 1 < S_TILES else None

                    # -------- attention for q-tile j --------
                    attnT = attp.tile([P, HPC, QW], BF16, tag="attnT", bufs=2)
                    for h in range(HPC):
                        kts = [
                            kt for kt in range(KT) if mask_table[(kt, j)] != "skip"
                        ]
                        pa = psum.tile([P, QW], F32, tag="pv", bufs=1)
                        den = psum.tile([P, QW], F32, tag="stat", bufs=1)
                        n_k = len(kts)
                        exs = [None] * n_k

                        def _den_pv(i):
                            kt = kts[i]
                            nc.tensor.matmul(
                                den[:], ones[:], exs[i][:],
                                start=(i == 0), stop=(i == n_k - 1),
                            )
                            nc.tensor.matmul(
                                pa[:],
                                vN[:, kt, ts(h, HD)],
                                exs[i][:],
                                start=(i == 0),
                                stop=(i == n_k - 1),
                            )

                        for i, kt in enumerate(kts):
                            msc = psum.tile([P, QW], F32, tag="score", bufs=3)
                            mt = mask_table[(kt, j)]
                            if mt != "plain":
                                # additive mask via PE accumulation: PSUM
                                # starts as the mask, score accumulates on top
                                nc.tensor.matmul(
                                    msc[:], ident_b[:], mtiles[:, mt, :],
                                    start=True, stop=False,
                                )
                            nc.tensor.matmul(
                                msc[:],
                                kT[:, h, ts(kt, P)],
                                qTs[:, h, :],
                                start=(mt == "plain"),
                                stop=True,
                            )
                            ex = expp.tile([P, QW], BF16, tag="exp")
                            nc.scalar.activation(ex[:], msc[:], AF.Exp)
                            exs[i] = ex
                            if i > 0:
                                _den_pv(i - 1)
                        _den_pv(n_k - 1)
                        rec = stats.tile([P, QW], F32, tag="rec")
                        nc.vector.reciprocal_approx_fast(rec[:], den[:])
                        nc.vector.tensor_mul(attnT[:, h, :], pa[:], rec[:])

                    if cs_next is not None:
                        rinv_next = stats_fin(cs_next)

                    # ---- wo partial + x/8 -> fp16 token-scattered RS in ----
                    g, r = chunk_of[(b, j)]
                    nsl = N_CORES // len(CHUNKS[g][1])  # slices from this j
                    tw = QW // nsl  # tokens per slice from this j-tile
                    for oq in range(C // 4):
                        # quad tile laid out (slice, chunk, token) so the DMA
                        # out AP collapses to 3 dims (c,t contiguous in DRAM)
                        ev = evp.tile([P, nsl, 4, tw], F16, tag="ev")
                        for oi in range(4):
                            ot = oq * 4 + oi
                            po = psum.tile([P, QW], F32, tag="mm", bufs=3)
                            for oc in range(OC):
                                nc.tensor.matmul(
                                    po[:],
                                    wo_sb[:, oc, ts(ot, P)],
                                    attnT[:, oc, :],
                                    start=(oc == 0),
                                    stop=(oc == OC - 1),
                                )
                            nc.vector.scalar_tensor_tensor(
                                ev[:, :, oi, :],
                                xb[:, ot, :].rearrange("p (s t) -> p s t", s=nsl),
                                1.0 / N_CORES,
                                po[:].rearrange("p (s t) -> p s t", s=nsl),
                                op0=ALU.mult, op1=ALU.add,
                            )
                        nc.sync.dma_start(
                            rs_in[g].ap()[
                                nsl * r : nsl * (r + 1), :, ts(oq, 4), :
                            ].rearrange("s p c t -> p s c t"),
                            ev[:],
                        )
                    if (b, j) == (CHUNKS[g][0], CHUNKS[g][1][-1]):
                        # chunk complete
                        if g == 1:
                            pending_rs = g  # emit after b1's x prefetch
                        else:
                            emit_rs(g)

        # ---------------- FFN phase (data-parallel, 512 tokens/core) --------
        with tc.tile_pool(name="hf", bufs=1) as hfp, \
             tc.tile_pool(name="w1p", bufs=2) as w1p, \
             tc.tile_pool(name="w2p", bufs=2) as w2p, \
             tc.tile_pool(name="up", bufs=1) as upp, \
             tc.tile_pool(name="oev", bufs=3) as oevp:
            # prefetch first w1 chunk BEFORE the RS-gated h loads so the
            # sync queue doesn't stall the weight stream behind RS-4.
            w1cs = [None] * W1CH
            w1c0 = w1p.tile([P, C, QW], BF16, tag="w1", name="w1c0")
            w1cs[0] = w1c0
            nc.sync.dma_start(w1cs[0][:], w1h[0])

            hf = hfp.tile([P, C, QW], F16, tag="hf")
            for g in range(G):
                nc.sync.dma_start(hf[:, :, col0[g] : col0[g] + slw[g]], rs_out[g].ap())

            # rmsnorm2 stats (consumed only at the down outputs; the PE
            # never waits: relu/down are scale-deferred)
            cs = psum.tile([P, QW], F32, tag="mm", bufs=3)
            for c in range(C):
                sq = sqp.tile([P, QW], BF16, tag="sq")
                nc.vector.tensor_mul(sq[:], hf[:, c, :], hf[:, c, :])
                nc.tensor.matmul(
                    cs[:], ones[:], sq[:], start=(c == 0), stop=(c == C - 1)
                )
            rms2 = stats.tile([P, QW], F32, tag="rms")
            nc.scalar.activation(
                rms2[:], cs[:], AF.Sqrt, bias=eps_p1[:], scale=1.0 / D
            )
            r2 = stats.tile([P, QW], F32, tag="rinv")
            nc.vector.reciprocal_approx_fast(r2[:], rms2[:])

            up_sb = upp.tile([P, HIDC, QW], BF16, tag="up")
            for ch in range(W1CH):
                if w1cs[ch] is None:
                    w1cn = w1p.tile([P, C, QW], BF16, tag="w1", name=f"w1c{ch}")
                    w1cs[ch] = w1cn
                    nc.sync.dma_start(w1cs[ch][:], w1h[ch])
                w1c = w1cs[ch]
                for hti in range(HTPC):
                    pu = psum.tile([P, QW], F32, tag="mm", bufs=3)
                    for c in range(C):
                        nc.tensor.matmul(
                            pu[:],
                            w1c[:, c, ts(hti, P)],
                            hf[:, c, :],
                            start=(c == 0),
                            stop=(c == C - 1),
                        )
                    nc.scalar.activation(
                        up_sb[:, ch * HTPC + hti, :], pu[:], AF.Relu
                    )

            for ot in range(C):
                w2c = w2p.tile([P, HIDC, P], BF16, tag="w2")
                nc.sync.dma_start(w2c[:], w2h[ot])
                pd = psum.tile([P, QW], F32, tag="mm", bufs=3)
                for hc in range(HIDC):
                    nc.tensor.matmul(
                        pd[:],
                        w2c[:, hc, :],
                        up_sb[:, hc, :],
                        start=(hc == 0),
                        stop=(hc == HIDC - 1),
                    )
                dn = oevp.tile([P, QW], F32, tag="dn")
                nc.vector.tensor_mul(dn[:], pd[:], r2[:])
                oev = oevp.tile([P, QW], F32, tag="oev")
                nc.vector.tensor_add(oev[:], hf[:, ot, :], dn[:])
                nc.sync.dma_start(out[ot], oev[:])

    nc.compile()
    return nc


_CACHE = {}
LAST_RESULT = None


def _get_program(B, S, D, H, HID, mask_table, n_mask, mask_key):
    key = (B, S, D, H, HID, mask_key)
    if key not in _CACHE:
        _CACHE[key] = build_program(B, S, D, H, HID, mask_table, n_mask)
    return _CACHE[key]


def kernel(x, mask, wq, wk, wv, wo, w1, w2, attn_norm_w, ffn_norm_w):
    x = np.asarray(x, dtype=np.float32)
    mask = np.asarray(mask, dtype=np.float32)
    wq, wk, wv, wo = (np.asarray(a, dtype=np.float32) for a in (wq, wk, wv, wo))
    w1, w2 = np.asarray(w1, dtype=np.float32), np.asarray(w2, dtype=np.float32)
    attn_norm_w = np.asarray(attn_norm_w, dtype=np.float32)
    ffn_norm_w = np.asarray(ffn_norm_w, dtype=np.float32)

    B, S, D = x.shape
    H = D // 128  # HD is fixed at 128 (= SBUF partition count)
    HID = w1.shape[0]
    HD = D // H
    HPC = H // N_CORES
    C = D // P
    HIDC = HID // P
    W1CH = 16

    mask_table, mtiles_np = _classify_mask(
        np.broadcast_to(mask, (1, 1, S, S))[0, 0], S
    )
    mask_key = hash(tuple(sorted((k, str(v)) for k, v in mask_table.items())))
    nc = _get_program(B, S, D, H, HID, mask_table, len(mtiles_np), mask_key)

    # ---- host-side prep ----
    xt = np.ascontiguousarray(x.transpose(0, 2, 1))  # [B, D, S]
    wq_f = (wq * attn_norm_w[None, :]) / np.sqrt(HD)
    wk_f = wk * attn_norm_w[None, :]
    wv_f = wv * attn_norm_w[None, :]
    w1_f = w1 * ffn_norm_w[None, :]

    # full FFN weights, replicated on every core (data-parallel FFN)
    # w1h[ch, p, c, o] = w1_f[hid = ch*512 + o, d = c*128 + p]
    w1host = np.ascontiguousarray(
        w1_f.reshape(W1CH, QW, C, P).transpose(0, 3, 2, 1)
    ).astype(BF16_NP)
    # w2h[ot, p, hc, o] = w2[d_out = ot*128 + o, hid = hc*128 + p]
    w2host = np.ascontiguousarray(
        w2.reshape(C, P, HIDC, P).transpose(0, 3, 2, 1)
    ).astype(BF16_NP)

    in_maps = []
    for c in range(N_CORES):
        hs = slice(c * HPC * HD, (c + 1) * HPC * HD)
        qs = np.ascontiguousarray(wq_f[hs].T).reshape(C, P, HPC * HD).astype(BF16_NP)
        ks = np.ascontiguousarray(wk_f[hs].T).reshape(C, P, HPC * HD).astype(BF16_NP)
        vs = np.ascontiguousarray(wv_f[hs].T).reshape(C, P, HPC * HD).astype(BF16_NP)
        os_ = np.ascontiguousarray(wo[:, hs].T).reshape(HPC, P, D).astype(BF16_NP)
        m = {
            "xt": xt,
            "wq": qs,
            "wk": ks,
            "wv": vs,
            "wo": os_,
            "w1h": w1host,
            "w2h": w2host,
        }
        if len(mtiles_np):
            m["mk"] = np.stack(mtiles_np).astype(BF16_NP)
        in_maps.append(m)

    trace = os.environ.get("KTRACE", "0") == "1"
    res = run_bass_kernel_spmd(nc, in_maps, list(range(N_CORES)), trace=trace)
    global LAST_RESULT
    LAST_RESULT = res

    # out[c] is [C, P, QW] f32 = [D, 512]; FFN column ranges map back to
    # tokens per the chunk table.
    full = np.empty((B * S, D), dtype=np.float32)
    col = 0
    starts = []
    for b, js in CHUNKS:
        starts.append(col)
        col += QW * len(js) // N_CORES
    for core in range(N_CORES):
        o = res.results[core]["out"].reshape(D, QW)
        for g, (b, js) in enumerate(CHUNKS):
            sl = QW * len(js) // N_CORES
            if len(js) == 2:
                j = js[core // 4]
                tok0 = S * b + QW * j + sl * (core % 4)
            else:
                j = js[0]
                tok0 = S * b + QW * j + sl * core
            full[tok0 : tok0 + sl, :] = o[:, starts[g] : starts[g] + sl].T
    return np.ascontiguousarray(full.reshape(B, S, D))


# revision 12
# speedup vs baseline: 1.7342x; 1.0626x over previous
"""Trainium2 Bass kernel for a dense transformer decoder block.

Strategy (8 NeuronCores):
  - Attention tensor-parallel over heads (2 heads/core); activations in
    transposed layout [D, tokens]; all matmuls bf16 with fp32 PSUM.
  - Instead of ReduceScattering the wo *outputs* (16.8 MB), the per-head
    attention outputs are exchanged with chunked AllToAlls (2 MB total,
    bf16, DeepSpeed-Ulysses style): after the A2A each core holds all 16
    heads for its own 512 tokens and computes the full wo locally (same
    FLOPs, overlapped chunk-by-chunk with the remaining attention), adds
    the exact f32 x residual from a per-core xres input, and keeps h in
    SBUF. No reduction collective at all.
  - FFN is data-parallel: each core runs the FULL FFN on its 512 tokens,
    streaming the full w1/w2 (bf16) from HBM under the matmuls. rmsnorm
    scaling is deferred through relu and the (linear) down-projection
    (relu(r*u) = r*relu(u), r>0), applied once on the 16 down outputs,
    so the PE never waits on the stats chain. Each core's (h + down) IS
    the final output for its tokens.
  - Engine balance: squares on DVE, exp on ACT, reciprocals via the fast
    DVE approximation, masks added via PE accumulation; stats computed
    one q-tile ahead so the PE runs dense.
  - Causality is not hardcoded: the mask input is classified host-side
    into skip / plain / mixed 128x512 blocks; mixed tiles are shipped
    as constants (4 distinct tiles for a causal mask).
"""

import os
import sys

try:  # the axon sitecustomize usually provides concourse already
    import concourse.bass  # noqa: F401
except ImportError:  # pragma: no cover
    sys.path.insert(0, "/opt/trn_rl_repo")

from contextlib import ExitStack

import ml_dtypes
import numpy as np

import concourse.bacc as bacc
import concourse.tile as tile
from concourse import mybir
from concourse.bass_utils import run_bass_kernel_spmd
from concourse.masks import make_identity

F32 = mybir.dt.float32
BF16 = mybir.dt.bfloat16
F16 = mybir.dt.float16
N_CORES = 8
P = 128
QW = 512  # q-tile / token-tile width
EPS = 1e-6
AF = mybir.ActivationFunctionType
ALU = mybir.AluOpType
BF16_NP = ml_dtypes.bfloat16
G = 4  # A2A chunks (one per pair of q-tiles)
SL = 128  # tokens per core-slice per chunk


def ts(i, w):
    return slice(i * w, (i + 1) * w)


def _classify_mask(mask, S):
    """mask: [S, S] additive (q, k). Returns (table, tiles).
    table[(kt, j)] = 'skip' | 'plain' | int mask-tile index.
    tiles: list of [128, QW] float32 arrays in scoresT ([k, q]) layout."""
    table = {}
    tiles = []
    keys = {}
    for j in range(S // QW):
        for kt in range(S // P):
            sub = mask[ts(j, QW), ts(kt, P)]  # [q, k]
            if np.all(sub <= -1e8):
                table[(kt, j)] = "skip"
            elif np.all(sub == 0.0):
                table[(kt, j)] = "plain"
            else:
                t = np.ascontiguousarray(sub.T.astype(np.float32))  # [k, q]
                key = t.tobytes()
                if key not in keys:
                    keys[key] = len(tiles)
                    tiles.append(t)
                table[(kt, j)] = keys[key]
    return table, tiles


def build_program(B, S, D, H, HID, mask_table, n_mask):
    HD = 128
    assert D == (D // P) * P and H * HD == D
    HPC = H // N_CORES            # heads per core
    assert HPC * N_CORES == H
    C = D // P                    # contraction chunks over D
    S_TILES = S // QW             # q tiles per batch
    KT = S // P                   # k tiles per batch
    HIDC = HID // P               # hidden tiles total (64)
    W1CH = 16                     # w1 stream chunks
    HTPC = HIDC // W1CH           # hid tiles per w1 chunk (4)

    nc = bacc.Bacc(trn_type="TRN2", num_devices=N_CORES)

    xt = nc.dram_tensor("xt", [B, D, S], F32, kind="ExternalInput").ap()
    xres = nc.dram_tensor("xres", [G, P, C, SL], F32, kind="ExternalInput").ap()
    wq = nc.dram_tensor("wq", [C, P, HPC * HD], BF16, kind="ExternalInput").ap()
    wk = nc.dram_tensor("wk", [C, P, HPC * HD], BF16, kind="ExternalInput").ap()
    wv = nc.dram_tensor("wv", [C, P, HPC * HD], BF16, kind="ExternalInput").ap()
    woh = nc.dram_tensor("woh", [P, H, D], BF16, kind="ExternalInput").ap()
    w1h = nc.dram_tensor("w1h", [W1CH, P, C, QW], BF16, kind="ExternalInput").ap()
    w2h = nc.dram_tensor("w2h", [C, P, HIDC, P], BF16, kind="ExternalInput").ap()
    mk = None
    if n_mask:
        mk = nc.dram_tensor("mk", [n_mask, P, QW], BF16, kind="ExternalInput").ap()

    a2a_in = [
        nc.dram_tensor(f"a2a_in{g}", [N_CORES, HPC, P, SL], BF16)
        for g in range(G)
    ]
    a2a_out = [
        nc.dram_tensor(f"a2a_out{g}", [N_CORES, HPC, P, SL], BF16)
        for g in range(G)
    ]
    out = nc.dram_tensor("out", [C, P, QW], F32, kind="ExternalOutput").ap()

    groups = [list(range(N_CORES))]

    with tile.TileContext(nc) as tc, ExitStack() as ctx:
        const = ctx.enter_context(tc.tile_pool(name="const", bufs=1))
        stats = ctx.enter_context(tc.tile_pool(name="stats", bufs=2))
        sqp = ctx.enter_context(tc.tile_pool(name="sq", bufs=2))
        hfp = ctx.enter_context(tc.tile_pool(name="hf", bufs=1))
        psum = ctx.enter_context(tc.tile_pool(name="psum", bufs=1, space="PSUM"))

        ones_f32 = const.tile([P, P], F32)
        nc.vector.memset(ones_f32[:], 1.0)
        ones = const.tile([P, P], BF16)
        nc.vector.tensor_copy(ones[:], ones_f32[:])
        eps_p1 = const.tile([P, 1], F32)
        nc.vector.memset(eps_p1[:], EPS)
        ident = const.tile([P, P], F32)
        make_identity(nc, ident[:])
        ident_b = const.tile([P, P], BF16)
        nc.vector.tensor_copy(ident_b[:], ident[:])

        # h for this core's 512 tokens, assembled chunk by chunk
        hf = hfp.tile([P, C, QW], F16, tag="hf")

        # ---------------- attention phase ----------------
        with tc.tile_pool(name="wqkv", bufs=1) as wqkvp, \
             tc.tile_pool(name="xa", bufs=2) as xap, \
             tc.tile_pool(name="xr", bufs=1) as xrp, \
             tc.tile_pool(name="qkv", bufs=1) as qkvp, \
             tc.tile_pool(name="ao", bufs=2) as aop, \
             tc.tile_pool(name="exp", bufs=3) as expp, \
             tc.tile_pool(name="attn", bufs=1) as attp:
            wq_sb = wqkvp.tile([P, C, HPC * HD], BF16, tag="wq")
            nc.sync.dma_start(wq_sb[:], wq.rearrange("c p o -> p c o"))
            wk_sb = wqkvp.tile([P, C, HPC * HD], BF16, tag="wk")
            nc.sync.dma_start(wk_sb[:], wk.rearrange("c p o -> p c o"))
            wv_sb = wqkvp.tile([P, C, HPC * HD], BF16, tag="wv")
            nc.scalar.dma_start(wv_sb[:], wv.rearrange("c p o -> p c o"))
            mtiles = None
            if n_mask:
                mtiles = wqkvp.tile([P, n_mask, QW], BF16, tag="mk")
                nc.scalar.dma_start(mtiles[:], mk.rearrange("n p q -> p n q"))
            wo_sb = wqkvp.tile([P, H, D], BF16, tag="wo")
            nc.scalar.dma_start(wo_sb[:], woh)

            def emit_a2a(g):
                nc.gpsimd.collective_compute(
                    "AllToAll",
                    ALU.bypass,
                    replica_groups=groups,
                    ins=[a2a_in[g].ap().opt()],
                    outs=[a2a_out[g].ap().opt()],
                )

            def stats_mm(xb):
                """sq on DVE, column sums via PE."""
                cs = psum.tile([P, QW], F32, tag="mm", bufs=3)
                for c in range(C):
                    sq = sqp.tile([P, QW], BF16, tag="sq")
                    nc.vector.tensor_mul(sq[:], xb[:, c, :], xb[:, c, :])
                    nc.tensor.matmul(
                        cs[:], ones[:], sq[:], start=(c == 0), stop=(c == C - 1)
                    )
                return cs

            def stats_fin(cs):
                """sqrt on ACT (emitted outside the exp burst) + fast recip."""
                rms = stats.tile([P, QW], F32, tag="rms")
                nc.scalar.activation(
                    rms[:], cs[:], AF.Sqrt, bias=eps_p1[:], scale=1.0 / D
                )
                rinv = stats.tile([P, QW], F32, tag="rinv")
                nc.vector.reciprocal_approx_fast(rinv[:], rms[:])
                return rinv

            def emit_wo(g):
                """Local full-wo for this core's chunk-g tokens + x residual
                -> hf columns [g*SL, (g+1)*SL)."""
                ao = aop.tile([P, H, SL], BF16, tag="ao")
                nc.sync.dma_start(
                    ao[:], a2a_out[g].ap().rearrange("s h p t -> p (s h) t")
                )
                xr = xrp.tile([P, C, SL], F32, tag="xr")
                nc.sync.dma_start(xr[:], xres[g])
                for ot in range(C):
                    po = psum.tile([P, SL], F32, tag="mm", bufs=3)
                    for oc in range(H):
                        nc.tensor.matmul(
                            po[:],
                            wo_sb[:, oc, ts(ot, P)],
                            ao[:, oc, :],
                            start=(oc == 0),
                            stop=(oc == H - 1),
                        )
                    nc.vector.tensor_add(
                        hf[:, ot, ts(g, SL)], xr[:, ot, :], po[:]
                    )

            pending_a2a = None
            for b in range(B):
                # prefetch x tiles of this batch (bf16 via cast DMA) BEFORE
                # any pending collective blocks the gpsimd queue.
                xbs = []

                def load_x(j):
                    xb = xap.tile([P, C, QW], BF16, tag="xb", name=f"xb{b}_{j}")
                    nc.gpsimd.dma_start(
                        xb[:],
                        xt[b].rearrange("(c p) t -> p c t", p=P)[:, :, ts(j, QW)],
                    )
                    xbs.append(xb)

                load_x(0)
                load_x(1)
                # the pending collective's inputs are ready exactly when the
                # first two x-prefetch WARs clear; later x loads go behind it
                if pending_a2a is not None:
                    emit_a2a(pending_a2a)
                    pending_a2a = None
                load_x(2)
                load_x(3)

                rinv_next = stats_fin(stats_mm(xbs[0]))

                kT = qkvp.tile([P, HPC, S], BF16, tag="kT")
                vN = qkvp.tile([P, KT, HPC * HD], BF16, tag="vN")
                for j in range(S_TILES):
                    g = 2 * b + j // 2
                    r = j % 2
                    xb = xbs[j]
                    rinv = rinv_next
                    # q/k projections from RAW x; rinv folded in at eviction
                    qTs = qkvp.tile([P, HPC, QW], BF16, tag="qT", bufs=2)
                    for h in range(HPC):
                        for w_sb, dst in ((wq_sb, qTs), (wk_sb, kT)):
                            pp = psum.tile([P, QW], F32, tag="mm", bufs=3)
                            for c in range(C):
                                nc.tensor.matmul(
                                    pp[:],
                                    w_sb[:, c, ts(h, HD)],
                                    xb[:, c, :],
                                    start=(c == 0),
                                    stop=(c == C - 1),
                                )
                            if dst is qTs:
                                nc.vector.tensor_mul(qTs[:, h, :], pp[:], rinv[:])
                            else:
                                nc.vector.tensor_mul(
                                    kT[:, h, ts(j, QW)], pp[:], rinv[:]
                                )
                    # rinv transposed to token-partition layout for v scaling
                    rcol = stats.tile([P, QW // P], F32, tag="rcol")
                    for sub in range(QW // P):
                        tp = psum.tile([P, P], F32, tag="mm", bufs=3)
                        nc.tensor.transpose(tp[:], rinv[:, ts(sub, P)], ident[:])
                        nc.vector.tensor_copy(rcol[:, sub : sub + 1], tp[:, 0:1])
                    # v in natural layout; rinv via per-partition scalar
                    for sub in range(QW // P):
                        pv = psum.tile([P, QW], F32, tag="mm", bufs=3)
                        for c in range(C):
                            nc.tensor.matmul(
                                pv[:, : HPC * HD],
                                xb[:, c, ts(sub, P)],
                                wv_sb[:, c, :],
                                start=(c == 0),
                                stop=(c == C - 1),
                            )
                        nc.vector.tensor_scalar_mul(
                            vN[:, j * (QW // P) + sub, :],
                            pv[:, : HPC * HD],
                            rcol[:, sub : sub + 1],
                        )

                    # local wo for the PREVIOUS chunk while its A2A is in
                    # flight behind one q-tile of attention
                    if r == 0 and g > 0:
                        emit_wo(g - 1)

                    # stats matmuls for the NEXT q-tile (ACT sqrt emitted
                    # after the exp burst below)
                    cs_next = stats_mm(xbs[j + 1]) if j + 1 < S_TILES else None

                    # -------- attention for q-tile j --------
                    attnT = attp.tile([P, HPC, QW], BF16, tag="attnT", bufs=2)
                    for h in range(HPC):
                        kts = [
                            kt for kt in range(KT) if mask_table[(kt, j)] != "skip"
                        ]
                        pa = psum.tile([P, QW], F32, tag="pv", bufs=1)
                        den = psum.tile([P, QW], F32, tag="stat", bufs=1)
                        n_k = len(kts)
                        exs = [None] * n_k

                        def _den_pv(i):
                            kt = kts[i]
                            nc.tensor.matmul(
                                den[:], ones[:], exs[i][:],
                                start=(i == 0), stop=(i == n_k - 1),
                            )
                            nc.tensor.matmul(
                                pa[:],
                                vN[:, kt, ts(h, HD)],
                                exs[i][:],
                                start=(i == 0),
                                stop=(i == n_k - 1),
                            )

                        for i, kt in enumerate(kts):
                            msc = psum.tile([P, QW], F32, tag="score", bufs=3)
                            mt = mask_table[(kt, j)]
                            if mt != "plain":
                                # additive mask via PE accumulation: PSUM
                                # starts as the mask, score accumulates on top
                                nc.tensor.matmul(
                                    msc[:], ident_b[:], mtiles[:, mt, :],
                                    start=True, stop=False,
                                )
                            nc.tensor.matmul(
                                msc[:],
                                kT[:, h, ts(kt, P)],
                                qTs[:, h, :],
                                start=(mt == "plain"),
                                stop=True,
                            )
                            ex = expp.tile([P, QW], BF16, tag="exp")
                            nc.scalar.activation(ex[:], msc[:], AF.Exp)
                            exs[i] = ex
                            if i > 0:
                                _den_pv(i - 1)
                        _den_pv(n_k - 1)
                        rec = stats.tile([P, QW], F32, tag="rec")
                        nc.vector.reciprocal_approx_fast(rec[:], den[:])
                        nc.vector.tensor_mul(attnT[:, h, :], pa[:], rec[:])

                    if cs_next is not None:
                        rinv_next = stats_fin(cs_next)

                    # ---- ship attnT head-blocks into the A2A input ----
                    for h in range(HPC):
                        nc.sync.dma_start(
                            a2a_in[g].ap()[4 * r : 4 * r + 4, h, :, :].rearrange(
                                "s p t -> p s t"
                            ),
                            attnT[:, h, :],
                        )
                    if r == 1:
                        if g == 1:
                            pending_a2a = g  # emit after b1's x prefetch
                        else:
                            emit_a2a(g)
            emit_wo(G - 1)

        # ---------------- FFN phase (data-parallel, 512 tokens/core) --------
        with tc.tile_pool(name="w1p", bufs=2) as w1p, \
             tc.tile_pool(name="w2p", bufs=2) as w2p, \
             tc.tile_pool(name="up", bufs=1) as upp, \
             tc.tile_pool(name="oev", bufs=3) as oevp:
            w1cs = [None] * W1CH
            w1c0 = w1p.tile([P, C, QW], BF16, tag="w1", name="w1c0")
            w1cs[0] = w1c0
            nc.sync.dma_start(w1cs[0][:], w1h[0])

            # rmsnorm2 stats (consumed only at the down outputs; the PE
            # never waits: relu/down are scale-deferred)
            cs = psum.tile([P, QW], F32, tag="mm", bufs=3)
            for c in range(C):
                sq = sqp.tile([P, QW], BF16, tag="sq")
                nc.vector.tensor_mul(sq[:], hf[:, c, :], hf[:, c, :])
                nc.tensor.matmul(
                    cs[:], ones[:], sq[:], start=(c == 0), stop=(c == C - 1)
                )
            rms2 = stats.tile([P, QW], F32, tag="rms")
            nc.scalar.activation(
                rms2[:], cs[:], AF.Sqrt, bias=eps_p1[:], scale=1.0 / D
            )
            r2 = stats.tile([P, QW], F32, tag="rinv")
            nc.vector.reciprocal_approx_fast(r2[:], rms2[:])

            up_sb = upp.tile([P, HIDC, QW], BF16, tag="up")
            for ch in range(W1CH):
                if w1cs[ch] is None:
                    w1cn = w1p.tile([P, C, QW], BF16, tag="w1", name=f"w1c{ch}")
                    w1cs[ch] = w1cn
                    nc.sync.dma_start(w1cs[ch][:], w1h[ch])
                w1c = w1cs[ch]
                for hti in range(HTPC):
                    pu = psum.tile([P, QW], F32, tag="mm", bufs=3)
                    for c in range(C):
                        nc.tensor.matmul(
                            pu[:],
                            w1c[:, c, ts(hti, P)],
                            hf[:, c, :],
                            start=(c == 0),
                            stop=(c == C - 1),
                        )
                    nc.scalar.activation(
                        up_sb[:, ch * HTPC + hti, :], pu[:], AF.Relu
                    )

            for ot in range(C):
                w2c = w2p.tile([P, HIDC, P], BF16, tag="w2")
                nc.sync.dma_start(w2c[:], w2h[ot])
                pd = psum.tile([P, QW], F32, tag="mm", bufs=3)
                for hc in range(HIDC):
                    nc.tensor.matmul(
                        pd[:],
                        w2c[:, hc, :],
                        up_sb[:, hc, :],
                        start=(hc == 0),
                        stop=(hc == HIDC - 1),
                    )
                dn = oevp.tile([P, QW], F32, tag="dn")
                nc.vector.tensor_mul(dn[:], pd[:], r2[:])
                oev = oevp.tile([P, QW], F32, tag="oev")
                nc.vector.tensor_add(oev[:], hf[:, ot, :], dn[:])
                nc.sync.dma_start(out[ot], oev[:])

    nc.compile()
    return nc


_CACHE = {}
LAST_RESULT = None


def _get_program(B, S, D, H, HID, mask_table, n_mask, mask_key):
    key = (B, S, D, H, HID, mask_key)
    if key not in _CACHE:
        _CACHE[key] = build_program(B, S, D, H, HID, mask_table, n_mask)
    return _CACHE[key]


def _core_tokens(core, S):
    """Global token start for (core, chunk g) blocks of SL tokens."""
    toks = []
    for g in range(G):
        b = g // 2
        j = 2 * (g % 2) + core // 4
        toks.append(S * b + QW * j + SL * (core % 4))
    return toks


def kernel(x, mask, wq, wk, wv, wo, w1, w2, attn_norm_w, ffn_norm_w):
    x = np.asarray(x, dtype=np.float32)
    mask = np.asarray(mask, dtype=np.float32)
    wq, wk, wv, wo = (np.asarray(a, dtype=np.float32) for a in (wq, wk, wv, wo))
    w1, w2 = np.asarray(w1, dtype=np.float32), np.asarray(w2, dtype=np.float32)
    attn_norm_w = np.asarray(attn_norm_w, dtype=np.float32)
    ffn_norm_w = np.asarray(ffn_norm_w, dtype=np.float32)

    B, S, D = x.shape
    H = D // 128  # HD is fixed at 128 (= SBUF partition count)
    HID = w1.shape[0]
    HD = D // H
    HPC = H // N_CORES
    C = D // P
    HIDC = HID // P
    W1CH = 16

    mask_table, mtiles_np = _classify_mask(
        np.broadcast_to(mask, (1, 1, S, S))[0, 0], S
    )
    mask_key = hash(tuple(sorted((k, str(v)) for k, v in mask_table.items())))
    nc = _get_program(B, S, D, H, HID, mask_table, len(mtiles_np), mask_key)

    # ---- host-side prep ----
    xt = np.ascontiguousarray(x.transpose(0, 2, 1))  # [B, D, S]
    wq_f = (wq * attn_norm_w[None, :]) / np.sqrt(HD)
    wk_f = wk * attn_norm_w[None, :]
    wv_f = wv * attn_norm_w[None, :]
    w1_f = w1 * ffn_norm_w[None, :]

    # full weights, replicated on every core
    # woh[p, oc, o] = wo[o, oc*128 + p]
    wohost = np.ascontiguousarray(
        wo.T.reshape(H, P, D).transpose(1, 0, 2)
    ).astype(BF16_NP)
    # w1h[ch, p, c, o] = w1_f[hid = ch*512 + o, d = c*128 + p]
    w1host = np.ascontiguousarray(
        w1_f.reshape(W1CH, QW, C, P).transpose(0, 3, 2, 1)
    ).astype(BF16_NP)
    # w2h[ot, p, hc, o] = w2[d_out = ot*128 + o, hid = hc*128 + p]
    w2host = np.ascontiguousarray(
        w2.reshape(C, P, HIDC, P).transpose(0, 3, 2, 1)
    ).astype(BF16_NP)

    xf = x.reshape(B * S, D)
    in_maps = []
    for c in range(N_CORES):
        hs = slice(c * HPC * HD, (c + 1) * HPC * HD)
        qs = np.ascontiguousarray(wq_f[hs].T).reshape(C, P, HPC * HD).astype(BF16_NP)
        ks = np.ascontiguousarray(wk_f[hs].T).reshape(C, P, HPC * HD).astype(BF16_NP)
        vs = np.ascontiguousarray(wv_f[hs].T).reshape(C, P, HPC * HD).astype(BF16_NP)
        # xres[g, p, cc, t] = x[token(g) + t, cc*128 + p]
        xr = np.empty((G, P, C, SL), dtype=np.float32)
        for g, tok0 in enumerate(_core_tokens(c, S)):
            xr[g] = xf[tok0 : tok0 + SL, :].T.reshape(C, P, SL).transpose(1, 0, 2)
        m = {
            "xt": xt,
            "xres": xr,
            "wq": qs,
            "wk": ks,
            "wv": vs,
            "woh": wohost,
            "w1h": w1host,
            "w2h": w2host,
        }
        if len(mtiles_np):
            m["mk"] = np.stack(mtiles_np).astype(BF16_NP)
        in_maps.append(m)

    trace = os.environ.get("KTRACE", "0") == "1"
    res = run_bass_kernel_spmd(nc, in_maps, list(range(N_CORES)), trace=trace)
    global LAST_RESULT
    LAST_RESULT = res

    full = np.empty((B * S, D), dtype=np.float32)
    for core in range(N_CORES):
        o = res.results[core]["out"].reshape(D, QW)
        for g, tok0 in enumerate(_core_tokens(core, S)):
            full[tok0 : tok0 + SL, :] = o[:, ts(g, SL)].T
    return np.ascontiguousarray(full.reshape(B, S, D))
